# revision 10
# baseline (speedup 1.0000x reference)
"""SnakeHead Trainium2 kernel (fp16 matmul path, PE-side bilinear).

Model (per batch): bilinear-sample a [256,256,126] feature map at 1024
vertices, concat the (y,x) coords -> [1024,128], 1x1 conv to 512 + ReLU,
six dilated (1,3,9,9,3,1) kernel-3 conv1d layers 512->512 + ReLU, final
1x1 conv 512->2.

Strategy: data-parallel over batch, 2 batches per NeuronCore (16/8).
Per core the kernel is tensor-engine-bound (~250us of fp16 matmul at
1 col/cycle); everything else is organized to keep the PE fed:
  - gather indices and bilinear corner weights are computed on HOST
    (they only depend on vertices); the first indirect gather fires as
    soon as the 8KB index DMA lands.
  - fm is host-repacked to fp16 quad rows fm[r] = [r |0| r+1 |0| r+W |0|
    r+W+1 |0] (4x128 fp16 = 1KB per token descriptor, 2 zero pad cols
    per corner); one 128-descriptor indirect DMA per 128-token tile.
  - the bilinear combine AND the token->channel transpose are both done
    ON THE PE: for each token tile, 4 accumulating matmuls
    psum[c,t] += q_corner.T @ diag(w_corner) with host-built diagonal
    weight matrices. No vector/scalar-engine combine work at all.
  - the vertex-coord concat is a second accumulating matmul in layer 0
    (lhsT = w0 rows 126:128, rhs = host-supplied channel-major verts).
  - conv layers run batch-outer so batch 0's layer 1 starts while batch
    1 is still gathering; all matmuls fp16 with fp32 PSUM accumulate.
  - layer 3-6 weight DMAs ride the Pool SWDGE queue strictly behind the
    gathers; layer 1-2 prefetch early on the Sync HWDGE queue.
  - final 1x1 conv is fused into the last conv layer per (batch, slice)
    block with per-block output DMA, so only ~2us trails the last matmul.
"""

import numpy as np
from contextlib import ExitStack

import concourse.bass as bass
import concourse.bacc as bacc
import concourse.mybir as mybir
import concourse.tile as tile
from concourse.bass import IndirectOffsetOnAxis
from concourse.bass_utils import run_bass_kernel_spmd

P = 128
B, N, H, W, Cf, Ch = 16, 1024, 256, 256, 126, 512
NCORES = 8
BPC = B // NCORES          # batches per core
T = BPC * N                # tokens per core
D = Cf + 2                 # input channels to layer 0
DILS = (1, 3, 9, 9, 3, 1)
PAD = 16                   # halo >= max dilation (9)
SEG = PAD + N + PAD        # per-batch activation columns
NT = T // P                # 128-token tiles per core (16)
CB = Ch // P               # 128-channel blocks (4)
HALF = 512                 # matmul moving-dim tile (tokens)
NS = N // HALF             # 2 (token-tile slices per batch)
QR = 4 * P                 # quad row width (4 corners x 128, zero padded)

F32 = mybir.dt.float32
BF = mybir.dt.float16
I32 = mybir.dt.int32
AF = mybir.ActivationFunctionType
ALU = mybir.AluOpType


def build_program(reps=1, nlayers=6):
    nc = bacc.Bacc(trn_type="TRN2", target_bir_lowering=False)

    idx = nc.declare_dram_parameter("idx", [P, NT], I32, False)
    dd = nc.declare_dram_parameter("dd", [P, NT * 4 * P], BF, False)
    vT = nc.declare_dram_parameter("vT", [2, T], BF, False)
    fm = nc.declare_dram_parameter("fm", [BPC * H * W, QR], BF, False)
    w0 = nc.declare_dram_parameter("w0", [P, Ch], BF, False)
    w0v = nc.declare_dram_parameter("w0v", [2, Ch], BF, False)
    b0 = nc.declare_dram_parameter("b0", [P, CB], F32, False)
    ws = nc.declare_dram_parameter("ws", [6, P, 3 * CB * Ch], BF, False)
    bs = nc.declare_dram_parameter("bs", [P, 6 * CB], F32, False)
    woff = nc.declare_dram_parameter("woff", [P, CB * 2], BF, False)
    out = nc.declare_dram_parameter("out", [2, T], F32, True)

    with tile.TileContext(nc) as tc, ExitStack() as ctx:
        const = ctx.enter_context(tc.tile_pool(name="const", bufs=1))
        gpool = ctx.enter_context(tc.tile_pool(name="gpool", bufs=2))
        wpool = ctx.enter_context(tc.tile_pool(name="wpool", bufs=1))
        hpool = ctx.enter_context(tc.tile_pool(name="hpool", bufs=1))
        psum = ctx.enter_context(tc.tile_pool(name="psum", bufs=4, space="PSUM"))
        for _ in range(reps):
            _emit_body(nc, tc, const, gpool, wpool, hpool, psum,
                       idx, dd, vT, fm, w0, w0v, b0, ws, bs, woff, out, nlayers)

    nc.reset()
    nc.finalize()
    return nc


def _emit_body(nc, tc, const, gpool, wpool, hpool, psum,
               idx, dd, vT, fm, w0, w0v, b0, ws, bs, woff, out, nlayers=6):
    # ---- Sync HWDGE queue loads, critical-path order: the index tile
    # first, then the diagonal bilinear weights interleaved with the
    # layer-1 weights (dd chunk g is needed when gather g's data lands).
    idx_sb = const.tile([P, NT], I32)
    nc.sync.dma_start(out=idx_sb[:], in_=idx[:])
    dd_sb = const.tile([P, NT * 4 * P], BF)
    ndd = NT // 4
    dchunk = 4 * 4 * P
    wcur = [wpool.tile([P, 3 * CB * Ch], BF, name=f"wlayer{li}",
                       tag=f"wlayer{li}") for li in range(nlayers)]
    for g in range(2):
        nc.sync.dma_start(out=dd_sb[:, g * dchunk:(g + 1) * dchunk],
                          in_=dd[:, g * dchunk:(g + 1) * dchunk])
    if nlayers > 0:
        nc.sync.dma_start(out=wcur[0][:], in_=ws[0])
    for g in range(2, ndd):
        nc.sync.dma_start(out=dd_sb[:, g * dchunk:(g + 1) * dchunk],
                          in_=dd[:, g * dchunk:(g + 1) * dchunk])
    vT_sb = const.tile([2, T], BF)
    nc.sync.dma_start(out=vT_sb[:], in_=vT[:])
    w0_sb = const.tile([P, Ch], BF)
    nc.sync.dma_start(out=w0_sb[:], in_=w0[:])
    w0v_sb = const.tile([2, Ch], BF)
    nc.sync.dma_start(out=w0v_sb[:], in_=w0v[:])
    b0_sb = const.tile([P, CB], F32)
    nc.sync.dma_start(out=b0_sb[:], in_=b0[:])
    bs_sb = const.tile([P, 6 * CB], F32)
    nc.sync.dma_start(out=bs_sb[:], in_=bs[:])
    woff_sb = const.tile([P, CB * 2], BF)
    nc.sync.dma_start(out=woff_sb[:], in_=woff[:])
    if nlayers > 1:
        nc.sync.dma_start(out=wcur[1][:], in_=ws[1])

    # ---- activation halo buffers; pads zeroed on the (idle) DVE up front ----
    h = [[[hpool.tile([P, SEG], BF, name=f"h{g}_{ci}_{b}", tag=f"h{g}_{ci}_{b}")
           for b in range(BPC)] for ci in range(CB)] for g in range(2)]
    for g in range(2):
        for ci in range(CB):
            for b in range(BPC):
                nc.vector.memset(h[g][ci][b][:, 0:PAD], 0.0)
                nc.vector.memset(h[g][ci][b][:, PAD + N:SEG], 0.0)

    def evict(ei, dst, ps, bias_ap):
        """ReLU+bias PSUM->SBUF eviction, alternating Act / DVE engines."""
        if ei % 2 == 0:
            nc.scalar.activation(dst, ps, AF.Relu, bias=bias_ap)
        else:
            nc.vector.tensor_scalar(
                out=dst, in0=ps, scalar1=bias_ap, scalar2=0.0,
                op0=ALU.add, op1=ALU.max)

    ev = 0

    def emit_l0(g):
        nonlocal ev
        b, s = g // NS, g % NS
        for co in range(CB):
            ps = psum.tile([P, HALF], F32, tag="mm", name=f"mm0_{g}_{co}")
            nc.tensor.matmul(
                ps[:], lhsT=w0_sb[:, co * P:(co + 1) * P], rhs=x_in[g][:],
                start=True, stop=False)
            nc.tensor.matmul(
                ps[:], lhsT=w0v_sb[:, co * P:(co + 1) * P],
                rhs=vT_sb[:, g * HALF:(g + 1) * HALF],
                start=False, stop=True)
            evict(ev, h[0][co][b][:, PAD + s * HALF:PAD + (s + 1) * HALF],
                  ps[:], b0_sb[:, co:co + 1])
            ev += 1

    # ---- gather + PE-side bilinear into x_in [128ch, 512tok] x 4 ----
    # One indirect gather per 128-token tile (HW: one descriptor per
    # partition).  Four accumulating diag-weight matmuls turn the quad
    # into channel-major bilinear samples directly in PSUM; one copy per
    # 4-tile group evicts to SBUF.  L0 trails one group behind.
    x_in = [const.tile([P, HALF], BF, name=f"x_in{t}") for t in range(NT // 4)]
    tps = None
    for j in range(NT):
        rq = gpool.tile([P, QR], BF, name="quad", tag="quad", bufs=6)
        nc.gpsimd.indirect_dma_start(
            out=rq[:], out_offset=None, in_=fm[:],
            in_offset=IndirectOffsetOnAxis(ap=idx_sb[:, j:j + 1], axis=0))
        if j % 4 == 0:
            tps = psum.tile([P, HALF], F32, tag="tps", bufs=2,
                            name=f"tps{j // 4}")
        q = j % 4
        for corner in range(4):
            nc.tensor.matmul(
                tps[:, q * P:(q + 1) * P],
                lhsT=rq[:, corner * P:(corner + 1) * P],
                rhs=dd_sb[:, (j * 4 + corner) * P:(j * 4 + corner + 1) * P],
                start=(corner == 0), stop=(corner == 3))
        if j % 4 == 3:
            g = j // 4
            if g % 2 == 0:
                nc.scalar.copy(out=x_in[g][:], in_=tps[:])
            else:
                nc.vector.tensor_copy(out=x_in[g][:], in_=tps[:])
            if g >= 1:
                emit_l0(g - 1)
    emit_l0(NT // 4 - 1)

    # remaining layer weights: Pool SWDGE queue, strictly behind gathers
    for li in range(2, nlayers):
        nc.gpsimd.dma_start(out=wcur[li][:], in_=ws[li])

    # ---- 6 dilated conv layers, batch-outer; final 1x1 conv fused into
    # the last layer per (b, s) block with per-block output DMA ----
    taps = [(k, ci) for k in range(3) for ci in range(CB)]
    out_sb = [const.tile([2, HALF], F32, name=f"osb_{t}") for t in range(BPC * NS)]
    fin_q = []   # deferred final-conv blocks (software pipelining on PE)

    def emit_fin(li, b, s):
        nonlocal ev
        gout = (li + 1) % 2
        sl = slice(PAD + s * HALF, PAD + (s + 1) * HALF)
        psf = psum.tile([2, HALF], F32, tag="fin", bufs=2, name=f"fin_{b}_{s}")
        for ci in range(CB):
            nc.tensor.matmul(
                psf[:],
                lhsT=woff_sb[:, ci * 2:(ci + 1) * 2],
                rhs=h[gout][ci][b][:, sl],
                start=(ci == 0), stop=(ci == CB - 1))
        ot = out_sb[b * NS + s]
        if ev % 2 == 0:
            nc.scalar.copy(out=ot[:], in_=psf[:])
        else:
            nc.vector.tensor_copy(out=ot[:], in_=psf[:])
        ev += 1
        nc.sync.dma_start(
            out=out[:, b * N + s * HALF:b * N + (s + 1) * HALF], in_=ot[:])

    for li, dil in enumerate(DILS[:nlayers]):
        gin, gout = li % 2, (li + 1) % 2
        wt = wcur[li]
        last = li == nlayers - 1
        for b in range(BPC):
            for s in range(NS):
                for co in range(CB):
                    ps = psum.tile([P, HALF], F32, tag="mm",
                                   name=f"mm{li}_{b}_{s}_{co}")
                    for ki, (k, ci) in enumerate(taps):
                        col = (k * CB + ci) * Ch + co * P
                        off = PAD + s * HALF + (k - 1) * dil
                        nc.tensor.matmul(
                            ps[:],
                            lhsT=wt[:, col:col + P],
                            rhs=h[gin][ci][b][:, off:off + HALF],
                            start=(ki == 0), stop=(ki == 3 * CB - 1))
                    evict(ev, h[gout][co][b][:, PAD + s * HALF:PAD + (s + 1) * HALF],
                          ps[:], bs_sb[:, li * CB + co:li * CB + co + 1])
                    ev += 1
                if last:
                    fin_q.append((li, b, s))
                    if len(fin_q) > 1:
                        emit_fin(*fin_q.pop(0))
    while fin_q:
        emit_fin(*fin_q.pop(0))

    if nlayers == 0:
        for b in range(BPC):
            for s in range(NS):
                emit_fin(-1, b, s)


def shard_inputs(vertices, feature_map, w0, b0, ws, bs, w_off):
    """Build the per-core input maps (host-side repack + index precompute)."""
    F16N = np.float16
    vertices = np.asarray(vertices, np.float32)
    feature_map = np.asarray(feature_map, np.float32)
    w0r = np.ascontiguousarray(w0.reshape(D, Ch))
    w0m = w0r[:Cf + 2].astype(F16N)       # full [128, Ch]; rows 126:127 unused
    w0v = np.ascontiguousarray(w0r[Cf:Cf + 2]).astype(F16N)   # [2, Ch]
    b0r = np.ascontiguousarray(b0.reshape(CB, P).T, np.float32)
    wsr = np.ascontiguousarray(
        ws.reshape(6, 3, CB, P, Ch).transpose(0, 3, 1, 2, 4).reshape(6, P, 3 * CB * Ch)
    ).astype(F16N)
    bsr = np.ascontiguousarray(
        bs.reshape(6, CB, P).transpose(2, 0, 1).reshape(P, 6 * CB), np.float32)
    woffr = np.ascontiguousarray(
        w_off.reshape(CB, P, 2).transpose(1, 0, 2).reshape(P, CB * 2)).astype(F16N)

    ptok = np.arange(P)
    in_maps = []
    for c in range(NCORES):
        vb = vertices[c * BPC:(c + 1) * BPC]          # [BPC, N, 2]
        # channel-major verts for the layer-0 concat matmul
        vTr = np.ascontiguousarray(
            vb.reshape(BPC * N, 2).T).astype(F16N)    # [2, T]
        # gather indices + bilinear corner weights (f32 = device fp path)
        coords = (vb + np.float32(1.0)) * np.float32((H - 1) / 2.0)  # [BPC,N,2]
        c0 = np.clip(np.floor(coords).astype(np.int64), 0, H - 2)
        frac = coords - c0.astype(np.float32)
        ridx = (c0[..., 0] * W + c0[..., 1]
                + (np.arange(BPC, dtype=np.int64) * (H * W))[:, None])  # [BPC,N]
        idxr = np.ascontiguousarray(
            ridx.reshape(BPC, N // P, P).transpose(2, 0, 1).reshape(P, NT)
        ).astype(np.int32)
        wy, wx = frac[..., 0], frac[..., 1]
        w4 = np.stack([(1 - wy) * (1 - wx), (1 - wy) * wx,
                       wy * (1 - wx), wy * wx], axis=0)  # [4,BPC,N] corner wts
        w4 = w4.reshape(4, BPC, N // P, P).transpose(3, 1, 2, 0)  # [P,BPC,N//P,4]
        w4 = w4.reshape(P, NT, 4).astype(F16N)
        ddr = np.zeros((P, NT, 4, P), F16N)           # diag(w_corner) per tile
        ddr[ptok, :, :, ptok] = w4[ptok]
        ddr = ddr.reshape(P, NT * 4 * P)
        # quad-packed fp16 feature map, 128-padded corners:
        # row r = [r |00| r+1 |00| r+W |00| r+W+1 |00]
        fmb = feature_map[c * BPC:(c + 1) * BPC].reshape(BPC * H * W, Cf).astype(F16N)
        R = BPC * H * W
        fmp = np.zeros((R, QR), F16N)
        fmp[:, 0 * P:0 * P + Cf] = fmb
        fmp[:R - 1, 1 * P:1 * P + Cf] = fmb[1:]
        fmp[:R - W, 2 * P:2 * P + Cf] = fmb[W:]
        fmp[:R - W - 1, 3 * P:3 * P + Cf] = fmb[W + 1:]
        in_maps.append({
            "idx": idxr, "dd": ddr, "vT": vTr, "fm": fmp,
            "w0": w0m, "w0v": w0v, "b0": b0r, "ws": wsr, "bs": bsr,
            "woff": woffr,
        })
    return in_maps


def unshard_output(results):
    outs = []
    for r in results:
        o = np.asarray(r["out"])                       # [2, T] = [ch, b*N+n]
        outs.append(o.reshape(2, BPC, N).transpose(1, 2, 0))   # [BPC, N, 2]
    return np.concatenate(outs, axis=0).astype(np.float32)


_NC_CACHE = {}


def _get_program():
    if "nc" not in _NC_CACHE:
        _NC_CACHE["nc"] = build_program()
    return _NC_CACHE["nc"]


def run(inputs, trace=False):
    nc = _get_program()
    in_maps = shard_inputs(**inputs)
    res = run_bass_kernel_spmd(nc, in_maps, list(range(NCORES)), trace=trace)
    return unshard_output(res.results), res


def kernel(**inputs) -> np.ndarray:
    out, _ = run(inputs, trace=False)
    return out


# revision 12
# speedup vs baseline: 1.0193x; 1.0193x over previous
"""SnakeHead Trainium2 kernel (fp16 matmul path, pipelined head).

Model (per batch): bilinear-sample a [256,256,126] feature map at 1024
vertices, concat the (y,x) coords -> [1024,128], 1x1 conv to 512 + ReLU,
six dilated (1,3,9,9,3,1) kernel-3 conv1d layers 512->512 + ReLU, final
1x1 conv 512->2.

Strategy: data-parallel over batch, 2 batches per NeuronCore (16/8).
Per core the kernel is tensor-engine-bound (~250us of fp16 matmul at
1 col/cycle); everything else is organized to keep the PE fed:
  - gather indices + bilinear weights are computed on HOST (they depend
    only on vertices); the first indirect gather fires as soon as the
    8KB index DMA lands.  The identity (for PE transposes) also comes
    in by DMA so the Pool engine queue holds nothing but gathers.
  - fm is host-repacked to fp16 quad rows fm[r] = [r | r+1 | r+W | r+W+1]
    (1008B per token descriptor); one 128-descriptor indirect DMA per
    128-token tile (the HW takes one index per partition per DMA).
  - the bilinear combine is split across the Act engine (scale by the
    y/x weights) and the DVE (fused multiply-add), in fp16, and the
    per-tile chains are SOFTWARE PIPELINED with one tile of skew so
    neither engine's in-order queue ever head-blocks on the other.
  - layer 0 runs per 512-token gather group; conv layers run batch-outer
    so batch 0's layer 1 starts while batch 1 is still gathering.
  - all matmuls fp16 (weights cast on host, fp32 PSUM accumulate).
  - final 1x1 conv is fused into the last conv layer per (batch, slice)
    block with per-block output DMA; the very last block interleaves
    the final-conv matmuls with the evictions so only ~2us trails the
    last conv matmul.
"""

import numpy as np
from contextlib import ExitStack

import concourse.bass as bass
import concourse.bacc as bacc
import concourse.mybir as mybir
import concourse.tile as tile
from concourse.bass import IndirectOffsetOnAxis
from concourse.bass_utils import run_bass_kernel_spmd

P = 128
B, N, H, W, Cf, Ch = 16, 1024, 256, 256, 126, 512
NCORES = 8
BPC = B // NCORES          # batches per core
T = BPC * N                # tokens per core
D = Cf + 2                 # input channels to layer 0
DILS = (1, 3, 9, 9, 3, 1)
PAD = 16                   # halo >= max dilation (9)
SEG = PAD + N + PAD        # per-batch activation columns
NT = T // P                # 128-token tiles per core (16)
CB = Ch // P               # 128-channel blocks (4)
HALF = 512                 # matmul moving-dim tile (tokens)
NS = N // HALF             # 2 (token-tile slices per batch)

F32 = mybir.dt.float32
BF = mybir.dt.float16
I32 = mybir.dt.int32
AF = mybir.ActivationFunctionType
ALU = mybir.AluOpType


def build_program(reps=1, nlayers=6):
    nc = bacc.Bacc(trn_type="TRN2", target_bir_lowering=False)

    idx = nc.declare_dram_parameter("idx", [P, NT], I32, False)
    uv = nc.declare_dram_parameter("uv", [P, 4 * NT], F32, False)
    verts = nc.declare_dram_parameter("verts", [P, NT * 2], F32, False)
    ident = nc.declare_dram_parameter("ident", [P, P], F32, False)
    fm = nc.declare_dram_parameter("fm", [BPC * H * W, 4 * Cf], BF, False)
    w0 = nc.declare_dram_parameter("w0", [P, Ch], BF, False)
    b0 = nc.declare_dram_parameter("b0", [P, CB], F32, False)
    ws = nc.declare_dram_parameter("ws", [6, P, 3 * CB * Ch], BF, False)
    bs = nc.declare_dram_parameter("bs", [P, 6 * CB], F32, False)
    woff = nc.declare_dram_parameter("woff", [P, CB * 2], BF, False)
    out = nc.declare_dram_parameter("out", [2, T], F32, True)

    with tile.TileContext(nc) as tc, ExitStack() as ctx:
        const = ctx.enter_context(tc.tile_pool(name="const", bufs=1))
        gpool = ctx.enter_context(tc.tile_pool(name="gpool", bufs=2))
        wpool = ctx.enter_context(tc.tile_pool(name="wpool", bufs=1))
        hpool = ctx.enter_context(tc.tile_pool(name="hpool", bufs=1))
        psum = ctx.enter_context(tc.tile_pool(name="psum", bufs=4, space="PSUM"))
        for _ in range(reps):
            _emit_body(nc, tc, const, gpool, wpool, hpool, psum,
                       idx, uv, verts, ident, fm, w0, b0, ws, bs, woff, out,
                       nlayers)

    nc.reset()
    nc.finalize()
    return nc


def _emit_body(nc, tc, const, gpool, wpool, hpool, psum,
               idx, uv, verts, ident, fm, w0, b0, ws, bs, woff, out,
               nlayers=6):
    # ---- Sync HWDGE queue loads, critical-path order ----
    idx_sb = const.tile([P, NT], I32)
    nc.sync.dma_start(out=idx_sb[:], in_=idx[:])
    uv_sb = const.tile([P, 4 * NT], F32)
    nc.sync.dma_start(out=uv_sb[:], in_=uv[:])
    v_sb = const.tile([P, NT * 2], F32)
    nc.sync.dma_start(out=v_sb[:], in_=verts[:])
    id_sb = const.tile([P, P], F32)
    nc.sync.dma_start(out=id_sb[:], in_=ident[:])
    wcur = [wpool.tile([P, 3 * CB * Ch], BF, name=f"wlayer{li}",
                       tag=f"wlayer{li}") for li in range(nlayers)]
    if nlayers > 0:
        nc.sync.dma_start(out=wcur[0][:], in_=ws[0])
    w0_sb = const.tile([P, Ch], BF)
    nc.sync.dma_start(out=w0_sb[:], in_=w0[:])
    b0_sb = const.tile([P, CB], F32)
    nc.sync.dma_start(out=b0_sb[:], in_=b0[:])
    bs_sb = const.tile([P, 6 * CB], F32)
    nc.sync.dma_start(out=bs_sb[:], in_=bs[:])
    woff_sb = const.tile([P, CB * 2], BF)
    nc.sync.dma_start(out=woff_sb[:], in_=woff[:])
    for li in range(1, nlayers):
        nc.sync.dma_start(out=wcur[li][:], in_=ws[li])

    # ---- activation halo buffers; pads zeroed on the (idle) DVE up front ----
    h = [[[hpool.tile([P, SEG], BF, name=f"h{g}_{ci}_{b}", tag=f"h{g}_{ci}_{b}")
           for b in range(BPC)] for ci in range(CB)] for g in range(2)]
    for g in range(2):
        for ci in range(CB):
            for b in range(BPC):
                nc.vector.memset(h[g][ci][b][:, 0:PAD], 0.0)
                nc.vector.memset(h[g][ci][b][:, PAD + N:SEG], 0.0)

    v3 = v_sb[:].rearrange("p (j t) -> p j t", t=2)       # [128, 16, 2]

    def evict(ei, dst, ps, bias_ap):
        """ReLU+bias PSUM->SBUF eviction, alternating Act / DVE engines."""
        if ei % 2 == 0:
            nc.scalar.activation(dst, ps, AF.Relu, bias=bias_ap)
        else:
            nc.vector.tensor_scalar(
                out=dst, in0=ps, scalar1=bias_ap, scalar2=0.0,
                op0=ALU.add, op1=ALU.max)

    ev = 0

    def emit_l0(g):
        nonlocal ev
        b, s = g // NS, g % NS
        for co in range(CB):
            ps = psum.tile([P, HALF], F32, tag="mm", name=f"mm0_{g}_{co}")
            nc.tensor.matmul(
                ps[:], lhsT=w0_sb[:, co * P:(co + 1) * P], rhs=x_in[g][:],
                start=True, stop=True)
            evict(ev, h[0][co][b][:, PAD + s * HALF:PAD + (s + 1) * HALF],
                  ps[:], b0_sb[:, co:co + 1])
            ev += 1

    # ---- gather + bilinear combine + transpose into x_in, software
    # pipelined: stage A(j) = gather + Act-scale + DVE-fma of the y
    # interp; stage B(j) (emitted one tile later) = x interp + verts col
    # + PE transpose; stage C(j) (two tiles later) = PSUM->x_in copy.
    # L0 for group g follows copy(4g+3); everything trails so no engine
    # queue head ever waits on a same-queue successor's dependency.
    x_in = [const.tile([P, HALF], BF, name=f"x_in{t}") for t in range(NT // 4)]
    tmps = [None] * NT
    tps_ = [None] * NT

    def stage_a(j):
        rq = gpool.tile([P, 4 * Cf], BF, name="quad", tag="quad", bufs=6)
        nc.gpsimd.indirect_dma_start(
            out=rq[:], out_offset=None, in_=fm[:],
            in_offset=IndirectOffsetOnAxis(ap=idx_sb[:, j:j + 1], axis=0))
        uy = uv_sb[:, 0 * NT + j:0 * NT + j + 1]
        wy = uv_sb[:, 1 * NT + j:1 * NT + j + 1]
        t1 = gpool.tile([P, 2 * Cf], BF, tag="t1", bufs=3)
        nc.scalar.mul(t1[:], rq[:, 0:2 * Cf], uy)
        tmp = gpool.tile([P, 2 * Cf], BF, tag="tmp", bufs=3)
        nc.vector.scalar_tensor_tensor(
            out=tmp[:], in0=rq[:, 2 * Cf:4 * Cf], scalar=wy, in1=t1[:],
            op0=ALU.mult, op1=ALU.add)
        tmps[j] = tmp

    def stage_b(j):
        tmp = tmps[j]
        ux = uv_sb[:, 2 * NT + j:2 * NT + j + 1]
        wx = uv_sb[:, 3 * NT + j:3 * NT + j + 1]
        t2 = gpool.tile([P, Cf], BF, tag="t2", bufs=3)
        nc.scalar.mul(t2[:], tmp[:, 0:Cf], ux)
        xpre = gpool.tile([P, P], F32, tag="xpre", bufs=3)
        nc.vector.scalar_tensor_tensor(
            out=xpre[:, 0:Cf], in0=tmp[:, Cf:2 * Cf], scalar=wx,
            in1=t2[:], op0=ALU.mult, op1=ALU.add)
        nc.vector.tensor_copy(out=xpre[:, Cf:Cf + 2], in_=v3[:, j, :])
        tp = psum.tile([P, P], F32, tag="tps", bufs=2, name=f"tp{j}")
        nc.tensor.transpose(out=tp[:], in_=xpre[:], identity=id_sb[:])
        tps_[j] = tp

    def stage_c(j):
        if j % 2 == 0:
            nc.scalar.copy(
                out=x_in[j // 4][:, (j % 4) * P:(j % 4 + 1) * P], in_=tps_[j][:])
        else:
            nc.vector.tensor_copy(
                out=x_in[j // 4][:, (j % 4) * P:(j % 4 + 1) * P], in_=tps_[j][:])
        if j % 4 == 3:
            emit_l0(j // 4)

    for j in range(NT):
        stage_a(j)
        if j >= 1:
            stage_b(j - 1)
        if j >= 2:
            stage_c(j - 2)
    stage_b(NT - 1)
    stage_c(NT - 2)
    stage_c(NT - 1)

    # ---- 6 dilated conv layers, batch-outer; final 1x1 conv fused into
    # the last layer per (b, s) block with per-block output DMA ----
    taps = [(k, ci) for k in range(3) for ci in range(CB)]
    out_sb = [const.tile([2, HALF], F32, name=f"osb_{t}") for t in range(BPC * NS)]
    fin_q = []   # deferred final-conv blocks (software pipelining on PE)

    def fin_dma(li, b, s, psf):
        nonlocal ev
        ot = out_sb[b * NS + s]
        if ev % 2 == 0:
            nc.scalar.copy(out=ot[:], in_=psf[:])
        else:
            nc.vector.tensor_copy(out=ot[:], in_=psf[:])
        ev += 1
        nc.sync.dma_start(
            out=out[:, b * N + s * HALF:b * N + (s + 1) * HALF], in_=ot[:])

    def emit_fin(li, b, s):
        gout = (li + 1) % 2
        sl = slice(PAD + s * HALF, PAD + (s + 1) * HALF)
        psf = psum.tile([2, HALF], F32, tag="fin", bufs=2, name=f"fin_{b}_{s}")
        for ci in range(CB):
            nc.tensor.matmul(
                psf[:],
                lhsT=woff_sb[:, ci * 2:(ci + 1) * 2],
                rhs=h[gout][ci][b][:, sl],
                start=(ci == 0), stop=(ci == CB - 1))
        fin_dma(li, b, s, psf)

    for li, dil in enumerate(DILS[:nlayers]):
        gin, gout = li % 2, (li + 1) % 2
        wt = wcur[li]
        last = li == nlayers - 1
        for b in range(BPC):
            for s in range(NS):
                last_blk = last and b == BPC - 1 and s == NS - 1
                psf = None
                if last_blk:
                    psf = psum.tile([2, HALF], F32, tag="fin", bufs=2,
                                    name="fin_last")
                for co in range(CB):
                    ps = psum.tile([P, HALF], F32, tag="mm",
                                   name=f"mm{li}_{b}_{s}_{co}")
                    for ki, (k, ci) in enumerate(taps):
                        col = (k * CB + ci) * Ch + co * P
                        off = PAD + s * HALF + (k - 1) * dil
                        nc.tensor.matmul(
                            ps[:],
                            lhsT=wt[:, col:col + P],
                            rhs=h[gin][ci][b][:, off:off + HALF],
                            start=(ki == 0), stop=(ki == 3 * CB - 1))
                    dst = h[gout][co][b][:, PAD + s * HALF:PAD + (s + 1) * HALF]
                    evict(ev, dst, ps[:], bs_sb[:, li * CB + co:li * CB + co + 1])
                    ev += 1
                    if last_blk:
                        # eager final conv: accumulate this channel block
                        # as soon as it is evicted -> ~2us tail, not ~4us
                        nc.tensor.matmul(
                            psf[:],
                            lhsT=woff_sb[:, co * 2:(co + 1) * 2],
                            rhs=dst,
                            start=(co == 0), stop=(co == CB - 1))
                if last_blk:
                    fin_dma(li, b, s, psf)
                elif last:
                    fin_q.append((li, b, s))
                    if len(fin_q) > 1:
                        emit_fin(*fin_q.pop(0))
    while fin_q:
        emit_fin(*fin_q.pop(0))

    if nlayers == 0:
        for b in range(BPC):
            for s in range(NS):
                emit_fin(-1, b, s)


def shard_inputs(vertices, feature_map, w0, b0, ws, bs, w_off):
    """Build the per-core input maps (host-side repack + index precompute)."""
    F16N = np.float16
    vertices = np.asarray(vertices, np.float32)
    feature_map = np.asarray(feature_map, np.float32)
    w0r = np.ascontiguousarray(w0.reshape(D, Ch)).astype(F16N)
    b0r = np.ascontiguousarray(b0.reshape(CB, P).T, np.float32)
    wsr = np.ascontiguousarray(
        ws.reshape(6, 3, CB, P, Ch).transpose(0, 3, 1, 2, 4).reshape(6, P, 3 * CB * Ch)
    ).astype(F16N)
    bsr = np.ascontiguousarray(
        bs.reshape(6, CB, P).transpose(2, 0, 1).reshape(P, 6 * CB), np.float32)
    woffr = np.ascontiguousarray(
        w_off.reshape(CB, P, 2).transpose(1, 0, 2).reshape(P, CB * 2)).astype(F16N)
    identity = np.eye(P, dtype=np.float32)

    in_maps = []
    for c in range(NCORES):
        vb = vertices[c * BPC:(c + 1) * BPC]          # [BPC, N, 2]
        vr = np.ascontiguousarray(
            vb.reshape(BPC, N // P, P, 2).transpose(2, 0, 1, 3).reshape(P, NT * 2))
        # gather indices + bilinear weights on host (f32 = device fp path)
        coords = (vb + np.float32(1.0)) * np.float32((H - 1) / 2.0)  # [BPC,N,2]
        c0 = np.clip(np.floor(coords).astype(np.int64), 0, H - 2)
        frac = coords - c0.astype(np.float32)
        ridx = (c0[..., 0] * W + c0[..., 1]
                + (np.arange(BPC, dtype=np.int64) * (H * W))[:, None])  # [BPC,N]
        idxr = np.ascontiguousarray(
            ridx.reshape(BPC, N // P, P).transpose(2, 0, 1).reshape(P, NT)
        ).astype(np.int32)
        wy, wx = frac[..., 0], frac[..., 1]
        uvr = np.stack([1.0 - wy, wy, 1.0 - wx, wx], axis=0)  # [4,BPC,N]
        uvr = np.ascontiguousarray(
            uvr.reshape(4, BPC, N // P, P).transpose(3, 0, 1, 2).reshape(P, 4 * NT)
        ).astype(np.float32)
        # quad-packed fp16 feature map: row r = [r | r+1 | r+W | r+W+1]
        fmb = feature_map[c * BPC:(c + 1) * BPC].reshape(BPC * H * W, Cf).astype(F16N)
        R = BPC * H * W
        fmp = np.zeros((R, 4 * Cf), F16N)
        fmp[:, 0 * Cf:1 * Cf] = fmb
        fmp[:R - 1, 1 * Cf:2 * Cf] = fmb[1:]
        fmp[:R - W, 2 * Cf:3 * Cf] = fmb[W:]
        fmp[:R - W - 1, 3 * Cf:4 * Cf] = fmb[W + 1:]
        in_maps.append({
            "idx": idxr, "uv": uvr, "verts": vr, "ident": identity, "fm": fmp,
            "w0": w0r, "b0": b0r, "ws": wsr, "bs": bsr, "woff": woffr,
        })
    return in_maps


def unshard_output(results):
    outs = []
    for r in results:
        o = np.asarray(r["out"])                       # [2, T] = [ch, b*N+n]
        outs.append(o.reshape(2, BPC, N).transpose(1, 2, 0))   # [BPC, N, 2]
    return np.concatenate(outs, axis=0).astype(np.float32)


_NC_CACHE = {}


def _get_program():
    if "nc" not in _NC_CACHE:
        _NC_CACHE["nc"] = build_program()
    return _NC_CACHE["nc"]


def run(inputs, trace=False):
    nc = _get_program()
    in_maps = shard_inputs(**inputs)
    res = run_bass_kernel_spmd(nc, in_maps, list(range(NCORES)), trace=trace)
    return unshard_output(res.results), res


def kernel(**inputs) -> np.ndarray:
    out, _ = run(inputs, trace=False)
    return out


# revision 15
# speedup vs baseline: 1.0503x; 1.0304x over previous
"""SnakeHead Trainium2 kernel (fp16 matmul path, pipelined head).

Model (per batch): bilinear-sample a [256,256,126] feature map at 1024
vertices, concat the (y,x) coords -> [1024,128], 1x1 conv to 512 + ReLU,
six dilated (1,3,9,9,3,1) kernel-3 conv1d layers 512->512 + ReLU, final
1x1 conv 512->2.

Strategy: data-parallel over batch, 2 batches per NeuronCore (16/8).
Per core the kernel is tensor-engine-bound (~250us of fp16 matmul at
1 col/cycle); everything else is organized to keep the PE fed:
  - gather indices + bilinear weights are computed on HOST (they depend
    only on vertices); the first indirect gather fires as soon as the
    8KB index DMA lands.  The identity (for PE transposes) also comes
    in by DMA so the Pool engine queue holds nothing but gathers.
  - fm is host-repacked to fp16 quad rows fm[r] = [r | r+1 | r+W | r+W+1]
    (1008B per token descriptor); one 128-descriptor indirect DMA per
    128-token tile (the HW takes one index per partition per DMA).
  - the bilinear combine is split across the Act engine (scale by the
    y/x weights) and the DVE (fused multiply-add), in fp16, and the
    per-tile chains are SOFTWARE PIPELINED with one tile of skew so
    neither engine's in-order queue ever head-blocks on the other.
  - layer 0 runs per 512-token gather group; conv layers run batch-outer
    so batch 0's layer 1 starts while batch 1 is still gathering.
  - all matmuls fp16 (weights cast on host, fp32 PSUM accumulate).
  - final 1x1 conv is fused into the last conv layer per (batch, slice)
    block with per-block output DMA; the very last block interleaves
    the final-conv matmuls with the evictions so only ~2us trails the
    last conv matmul.
"""

import numpy as np
from contextlib import ExitStack

import concourse.bass as bass
import concourse.bacc as bacc
import concourse.mybir as mybir
import concourse.tile as tile
from concourse.bass import IndirectOffsetOnAxis
from concourse.bass_utils import run_bass_kernel_spmd

P = 128
B, N, H, W, Cf, Ch = 16, 1024, 256, 256, 126, 512
NCORES = 8
BPC = B // NCORES          # batches per core
T = BPC * N                # tokens per core
D = Cf + 2                 # input channels to layer 0
DILS = (1, 3, 9, 9, 3, 1)
PAD = 16                   # halo >= max dilation (9)
SEG = PAD + N + PAD        # per-batch activation columns
NT = T // P                # 128-token tiles per core (16)
CB = Ch // P               # 128-channel blocks (4)
HALF = 512                 # matmul moving-dim tile (tokens)
NS = N // HALF             # 2 (token-tile slices per batch)

F32 = mybir.dt.float32
BF = mybir.dt.float16
I32 = mybir.dt.int32
AF = mybir.ActivationFunctionType
ALU = mybir.AluOpType


def build_program(reps=1, nlayers=6):
    nc = bacc.Bacc(trn_type="TRN2", target_bir_lowering=False)

    idx = nc.declare_dram_parameter("idx", [P, NT], I32, False)
    uv = nc.declare_dram_parameter("uv", [P, 4 * NT], F32, False)
    verts = nc.declare_dram_parameter("verts", [P, NT * 2], F32, False)
    ident = nc.declare_dram_parameter("ident", [P, P], F32, False)
    fm = nc.declare_dram_parameter("fm", [BPC * H * W, 4 * Cf], BF, False)
    w0 = nc.declare_dram_parameter("w0", [P, Ch], BF, False)
    b0 = nc.declare_dram_parameter("b0", [P, CB], F32, False)
    ws = nc.declare_dram_parameter("ws", [6, P, 3 * CB * Ch], BF, False)
    bs = nc.declare_dram_parameter("bs", [P, 6 * CB], F32, False)
    woff = nc.declare_dram_parameter("woff", [P, CB * 2], BF, False)
    out = nc.declare_dram_parameter("out", [2, T], F32, True)

    with tile.TileContext(nc) as tc, ExitStack() as ctx:
        const = ctx.enter_context(tc.tile_pool(name="const", bufs=1))
        gpool = ctx.enter_context(tc.tile_pool(name="gpool", bufs=2))
        wpool = ctx.enter_context(tc.tile_pool(name="wpool", bufs=1))
        hpool = ctx.enter_context(tc.tile_pool(name="hpool", bufs=1))
        psum = ctx.enter_context(tc.tile_pool(name="psum", bufs=4, space="PSUM"))
        for _ in range(reps):
            _emit_body(nc, tc, const, gpool, wpool, hpool, psum,
                       idx, uv, verts, ident, fm, w0, b0, ws, bs, woff, out,
                       nlayers)

    nc.reset()
    nc.finalize()
    return nc


def _emit_body(nc, tc, const, gpool, wpool, hpool, psum,
               idx, uv, verts, ident, fm, w0, b0, ws, bs, woff, out,
               nlayers=6):
    # ---- Sync HWDGE queue loads, critical-path order ----
    idx_sb = const.tile([P, NT], I32)
    nc.sync.dma_start(out=idx_sb[:], in_=idx[:])
    uv_sb = const.tile([P, 4 * NT], F32)
    nc.sync.dma_start(out=uv_sb[:], in_=uv[:])
    v_sb = const.tile([P, NT * 2], F32)
    nc.sync.dma_start(out=v_sb[:], in_=verts[:])
    id_sb = const.tile([P, P], F32)
    nc.sync.dma_start(out=id_sb[:], in_=ident[:])
    wcur = [wpool.tile([P, 3 * CB * Ch], BF, name=f"wlayer{li}",
                       tag=f"wlayer{li}") for li in range(nlayers)]
    w0_sb = const.tile([P, Ch], BF)
    nc.sync.dma_start(out=w0_sb[:], in_=w0[:])
    b0_sb = const.tile([P, CB], F32)
    nc.sync.dma_start(out=b0_sb[:], in_=b0[:])
    bs_sb = const.tile([P, 6 * CB], F32)
    nc.sync.dma_start(out=bs_sb[:], in_=bs[:])
    woff_sb = const.tile([P, CB * 2], BF)
    nc.sync.dma_start(out=woff_sb[:], in_=woff[:])
    # ws[0] is the only big weight DMA allowed to share HBM bandwidth
    # with the gathers (needed by layer 1 at ~18us); ws[1:] go through
    # the Pool SWDGE queue strictly BEHIND all 16 gathers (see below) so
    # they cannot starve the gather transfers.
    if nlayers > 0:
        nc.sync.dma_start(out=wcur[0][:], in_=ws[0])

    # ---- activation halo buffers; pads zeroed on the (idle) DVE up front ----
    h = [[[hpool.tile([P, SEG], BF, name=f"h{g}_{ci}_{b}", tag=f"h{g}_{ci}_{b}")
           for b in range(BPC)] for ci in range(CB)] for g in range(2)]
    for g in range(2):
        for ci in range(CB):
            for b in range(BPC):
                nc.vector.memset(h[g][ci][b][:, 0:PAD], 0.0)
                nc.vector.memset(h[g][ci][b][:, PAD + N:SEG], 0.0)

    v3 = v_sb[:].rearrange("p (j t) -> p j t", t=2)       # [128, 16, 2]

    def evict(ei, dst, ps, bias_ap):
        """ReLU+bias PSUM->SBUF eviction, alternating Act / DVE engines."""
        if ei % 2 == 0:
            nc.scalar.activation(dst, ps, AF.Relu, bias=bias_ap)
        else:
            nc.vector.tensor_scalar(
                out=dst, in0=ps, scalar1=bias_ap, scalar2=0.0,
                op0=ALU.add, op1=ALU.max)

    ev = 0

    def emit_l0(g):
        nonlocal ev
        b, s = g // NS, g % NS
        for co in range(CB):
            ps = psum.tile([P, HALF], F32, tag="mm", name=f"mm0_{g}_{co}")
            nc.tensor.matmul(
                ps[:], lhsT=w0_sb[:, co * P:(co + 1) * P], rhs=x_in[g][:],
                start=True, stop=True)
            evict(ev, h[0][co][b][:, PAD + s * HALF:PAD + (s + 1) * HALF],
                  ps[:], b0_sb[:, co:co + 1])
            ev += 1

    # ---- gather + bilinear combine + transpose into x_in, software
    # pipelined: stage A(j) = gather + Act-scale + DVE-fma of the y
    # interp; stage B(j) (emitted one tile later) = x interp + verts col
    # + PE transpose; stage C(j) (two tiles later) = PSUM->x_in copy.
    # L0 for group g follows copy(4g+3); everything trails so no engine
    # queue head ever waits on a same-queue successor's dependency.
    x_in = [const.tile([P, HALF], BF, name=f"x_in{t}") for t in range(NT // 4)]
    tmps = [None] * NT
    tps_ = [None] * NT

    def stage_a(j):
        rq = gpool.tile([P, 4 * Cf], BF, name="quad", tag="quad", bufs=16)
        nc.gpsimd.indirect_dma_start(
            out=rq[:], out_offset=None, in_=fm[:],
            in_offset=IndirectOffsetOnAxis(ap=idx_sb[:, j:j + 1], axis=0))
        uy = uv_sb[:, 0 * NT + j:0 * NT + j + 1]
        wy = uv_sb[:, 1 * NT + j:1 * NT + j + 1]
        t1 = gpool.tile([P, 2 * Cf], BF, tag="t1", bufs=3)
        nc.scalar.mul(t1[:], rq[:, 0:2 * Cf], uy)
        tmp = gpool.tile([P, 2 * Cf], BF, tag="tmp", bufs=3)
        nc.vector.scalar_tensor_tensor(
            out=tmp[:], in0=rq[:, 2 * Cf:4 * Cf], scalar=wy, in1=t1[:],
            op0=ALU.mult, op1=ALU.add)
        tmps[j] = tmp

    def stage_b(j):
        tmp = tmps[j]
        ux = uv_sb[:, 2 * NT + j:2 * NT + j + 1]
        wx = uv_sb[:, 3 * NT + j:3 * NT + j + 1]
        t2 = gpool.tile([P, Cf], BF, tag="t2", bufs=3)
        nc.scalar.mul(t2[:], tmp[:, 0:Cf], ux)
        xpre = gpool.tile([P, P], F32, tag="xpre", bufs=3)
        nc.vector.scalar_tensor_tensor(
            out=xpre[:, 0:Cf], in0=tmp[:, Cf:2 * Cf], scalar=wx,
            in1=t2[:], op0=ALU.mult, op1=ALU.add)
        nc.vector.tensor_copy(out=xpre[:, Cf:Cf + 2], in_=v3[:, j, :])
        tp = psum.tile([P, P], F32, tag="tps", bufs=2, name=f"tp{j}")
        nc.tensor.transpose(out=tp[:], in_=xpre[:], identity=id_sb[:])
        tps_[j] = tp

    def stage_c(j):
        if j % 2 == 0:
            nc.scalar.copy(
                out=x_in[j // 4][:, (j % 4) * P:(j % 4 + 1) * P], in_=tps_[j][:])
        else:
            nc.vector.tensor_copy(
                out=x_in[j // 4][:, (j % 4) * P:(j % 4 + 1) * P], in_=tps_[j][:])
        if j % 4 == 3:
            emit_l0(j // 4)

    for j in range(NT):
        stage_a(j)
        if j >= 1:
            stage_b(j - 1)
        if j >= 2:
            stage_c(j - 2)
    stage_b(NT - 1)
    stage_c(NT - 2)
    stage_c(NT - 1)

    # remaining layer weights: Pool SWDGE queue, strictly behind the
    # gathers so their 7.9MB cannot contend with the gather transfers
    for li in range(1, nlayers):
        nc.gpsimd.dma_start(out=wcur[li][:], in_=ws[li])

    # ---- 6 dilated conv layers, batch-outer; final 1x1 conv fused into
    # the last layer per (b, s) block with per-block output DMA ----
    taps = [(k, ci) for k in range(3) for ci in range(CB)]
    out_sb = [const.tile([2, HALF], F32, name=f"osb_{t}") for t in range(BPC * NS)]
    fin_q = []   # deferred final-conv blocks (software pipelining on PE)

    def fin_dma(li, b, s, psf):
        nonlocal ev
        ot = out_sb[b * NS + s]
        if ev % 2 == 0:
            nc.scalar.copy(out=ot[:], in_=psf[:])
        else:
            nc.vector.tensor_copy(out=ot[:], in_=psf[:])
        ev += 1
        nc.sync.dma_start(
            out=out[:, b * N + s * HALF:b * N + (s + 1) * HALF], in_=ot[:])

    def emit_fin(li, b, s):
        gout = (li + 1) % 2
        sl = slice(PAD + s * HALF, PAD + (s + 1) * HALF)
        psf = psum.tile([2, HALF], F32, tag="fin", bufs=2, name=f"fin_{b}_{s}")
        for ci in range(CB):
            nc.tensor.matmul(
                psf[:],
                lhsT=woff_sb[:, ci * 2:(ci + 1) * 2],
                rhs=h[gout][ci][b][:, sl],
                start=(ci == 0), stop=(ci == CB - 1))
        fin_dma(li, b, s, psf)

    for li, dil in enumerate(DILS[:nlayers]):
        gin, gout = li % 2, (li + 1) % 2
        wt = wcur[li]
        last = li == nlayers - 1
        for b in range(BPC):
            for s in range(NS):
                last_blk = last and b == BPC - 1 and s == NS - 1
                psf = None
                if last_blk:
                    psf = psum.tile([2, HALF], F32, tag="fin", bufs=2,
                                    name="fin_last")
                for co in range(CB):
                    ps = psum.tile([P, HALF], F32, tag="mm",
                                   name=f"mm{li}_{b}_{s}_{co}")
                    for ki, (k, ci) in enumerate(taps):
                        col = (k * CB + ci) * Ch + co * P
                        off = PAD + s * HALF + (k - 1) * dil
                        nc.tensor.matmul(
                            ps[:],
                            lhsT=wt[:, col:col + P],
                            rhs=h[gin][ci][b][:, off:off + HALF],
                            start=(ki == 0), stop=(ki == 3 * CB - 1))
                    dst = h[gout][co][b][:, PAD + s * HALF:PAD + (s + 1) * HALF]
                    evict(ev, dst, ps[:], bs_sb[:, li * CB + co:li * CB + co + 1])
                    ev += 1
                    if last_blk:
                        # eager final conv: accumulate this channel block
                        # as soon as it is evicted -> ~2us tail, not ~4us
                        nc.tensor.matmul(
                            psf[:],
                            lhsT=woff_sb[:, co * 2:(co + 1) * 2],
                            rhs=dst,
                            start=(co == 0), stop=(co == CB - 1))
                if last_blk:
                    fin_dma(li, b, s, psf)
                elif last:
                    fin_q.append((li, b, s))
                    if len(fin_q) > 1:
                        emit_fin(*fin_q.pop(0))
    while fin_q:
        emit_fin(*fin_q.pop(0))

    if nlayers == 0:
        for b in range(BPC):
            for s in range(NS):
                emit_fin(-1, b, s)


def shard_inputs(vertices, feature_map, w0, b0, ws, bs, w_off):
    """Build the per-core input maps (host-side repack + index precompute)."""
    F16N = np.float16
    vertices = np.asarray(vertices, np.float32)
    feature_map = np.asarray(feature_map, np.float32)
    w0r = np.ascontiguousarray(w0.reshape(D, Ch)).astype(F16N)
    b0r = np.ascontiguousarray(b0.reshape(CB, P).T, np.float32)
    wsr = np.ascontiguousarray(
        ws.reshape(6, 3, CB, P, Ch).transpose(0, 3, 1, 2, 4).reshape(6, P, 3 * CB * Ch)
    ).astype(F16N)
    bsr = np.ascontiguousarray(
        bs.reshape(6, CB, P).transpose(2, 0, 1).reshape(P, 6 * CB), np.float32)
    woffr = np.ascontiguousarray(
        w_off.reshape(CB, P, 2).transpose(1, 0, 2).reshape(P, CB * 2)).astype(F16N)
    identity = np.eye(P, dtype=np.float32)

    in_maps = []
    for c in range(NCORES):
        vb = vertices[c * BPC:(c + 1) * BPC]          # [BPC, N, 2]
        vr = np.ascontiguousarray(
            vb.reshape(BPC, N // P, P, 2).transpose(2, 0, 1, 3).reshape(P, NT * 2))
        # gather indices + bilinear weights on host (f32 = device fp path)
        coords = (vb + np.float32(1.0)) * np.float32((H - 1) / 2.0)  # [BPC,N,2]
        c0 = np.clip(np.floor(coords).astype(np.int64), 0, H - 2)
        frac = coords - c0.astype(np.float32)
        ridx = (c0[..., 0] * W + c0[..., 1]
                + (np.arange(BPC, dtype=np.int64) * (H * W))[:, None])  # [BPC,N]
        idxr = np.ascontiguousarray(
            ridx.reshape(BPC, N // P, P).transpose(2, 0, 1).reshape(P, NT)
        ).astype(np.int32)
        wy, wx = frac[..., 0], frac[..., 1]
        uvr = np.stack([1.0 - wy, wy, 1.0 - wx, wx], axis=0)  # [4,BPC,N]
        uvr = np.ascontiguousarray(
            uvr.reshape(4, BPC, N // P, P).transpose(3, 0, 1, 2).reshape(P, 4 * NT)
        ).astype(np.float32)
        # quad-packed fp16 feature map: row r = [r | r+1 | r+W | r+W+1]
        fmb = feature_map[c * BPC:(c + 1) * BPC].reshape(BPC * H * W, Cf).astype(F16N)
        R = BPC * H * W
        fmp = np.zeros((R, 4 * Cf), F16N)
        fmp[:, 0 * Cf:1 * Cf] = fmb
        fmp[:R - 1, 1 * Cf:2 * Cf] = fmb[1:]
        fmp[:R - W, 2 * Cf:3 * Cf] = fmb[W:]
        fmp[:R - W - 1, 3 * Cf:4 * Cf] = fmb[W + 1:]
        in_maps.append({
            "idx": idxr, "uv": uvr, "verts": vr, "ident": identity, "fm": fmp,
            "w0": w0r, "b0": b0r, "ws": wsr, "bs": bsr, "woff": woffr,
        })
    return in_maps


def unshard_output(results):
    outs = []
    for r in results:
        o = np.asarray(r["out"])                       # [2, T] = [ch, b*N+n]
        outs.append(o.reshape(2, BPC, N).transpose(1, 2, 0))   # [BPC, N, 2]
    return np.concatenate(outs, axis=0).astype(np.float32)


_NC_CACHE = {}


def _get_program():
    if "nc" not in _NC_CACHE:
        _NC_CACHE["nc"] = build_program()
    return _NC_CACHE["nc"]


def run(inputs, trace=False):
    nc = _get_program()
    in_maps = shard_inputs(**inputs)
    res = run_bass_kernel_spmd(nc, in_maps, list(range(NCORES)), trace=trace)
    return unshard_output(res.results), res


def kernel(**inputs) -> np.ndarray:
    out, _ = run(inputs, trace=False)
    return out


# revision 18
# speedup vs baseline: 1.0576x; 1.0069x over previous
"""SnakeHead Trainium2 kernel (fp16 matmul path, pipelined head).

Model (per batch): bilinear-sample a [256,256,126] feature map at 1024
vertices, concat the (y,x) coords -> [1024,128], 1x1 conv to 512 + ReLU,
six dilated (1,3,9,9,3,1) kernel-3 conv1d layers 512->512 + ReLU, final
1x1 conv 512->2.

Strategy: data-parallel over batch, 2 batches per NeuronCore (16/8).
Per core the kernel is tensor-engine-bound (~250us of fp16 matmul at
1 col/cycle); everything else is organized to keep the PE fed:
  - gather indices + bilinear weights are computed on HOST (they depend
    only on vertices); the first indirect gather fires as soon as the
    8KB index DMA lands.  The identity (for PE transposes) also comes
    in by DMA so the Pool engine queue holds nothing but gathers.
  - fm is host-repacked to fp16 quad rows fm[r] = [r | r+1 | r+W | r+W+1]
    (1008B per token descriptor); one 128-descriptor indirect DMA per
    128-token tile (the HW takes one index per partition per DMA).
  - the bilinear combine is split across the Act engine (scale by the
    y/x weights) and the DVE (fused multiply-add), in fp16, and the
    per-tile chains are SOFTWARE PIPELINED with one tile of skew so
    neither engine's in-order queue ever head-blocks on the other.
  - layer 0 runs per 512-token gather group; conv layers run batch-outer
    so batch 0's layer 1 starts while batch 1 is still gathering.
  - all matmuls fp16 (weights cast on host, fp32 PSUM accumulate).
  - final 1x1 conv is fused into the last conv layer per (batch, slice)
    block with per-block output DMA; the very last block interleaves
    the final-conv matmuls with the evictions so only ~2us trails the
    last conv matmul.
"""

import numpy as np
from contextlib import ExitStack

import concourse.bass as bass
import concourse.bacc as bacc
import concourse.mybir as mybir
import concourse.tile as tile
from concourse.bass import IndirectOffsetOnAxis
from concourse.bass_utils import run_bass_kernel_spmd

P = 128
B, N, H, W, Cf, Ch = 16, 1024, 256, 256, 126, 512
NCORES = 8
BPC = B // NCORES          # batches per core
T = BPC * N                # tokens per core
D = Cf + 2                 # input channels to layer 0
DILS = (1, 3, 9, 9, 3, 1)
PAD = 16                   # halo >= max dilation (9)
SEG = PAD + N + PAD        # per-batch activation columns
NT = T // P                # 128-token tiles per core (16)
CB = Ch // P               # 128-channel blocks (4)
HALF = 512                 # matmul moving-dim tile (tokens)
NS = N // HALF             # 2 (token-tile slices per batch)

F32 = mybir.dt.float32
BF = mybir.dt.float16
I32 = mybir.dt.int32
AF = mybir.ActivationFunctionType
ALU = mybir.AluOpType


def build_program(reps=1, nlayers=6):
    nc = bacc.Bacc(trn_type="TRN2", target_bir_lowering=False)

    idx = nc.declare_dram_parameter("idx", [P, NT], I32, False)
    uv = nc.declare_dram_parameter("uv", [P, 4 * NT], F32, False)
    verts = nc.declare_dram_parameter("verts", [P, NT * 2], F32, False)
    ident = nc.declare_dram_parameter("ident", [P, P], F32, False)
    fm = nc.declare_dram_parameter("fm", [BPC * H * W, 4 * Cf], BF, False)
    w0 = nc.declare_dram_parameter("w0", [P, Ch], BF, False)
    b0 = nc.declare_dram_parameter("b0", [P, CB], F32, False)
    ws = nc.declare_dram_parameter("ws", [6, P, 3 * CB * Ch], BF, False)
    bs = nc.declare_dram_parameter("bs", [P, 6 * CB], F32, False)
    woff = nc.declare_dram_parameter("woff", [P, CB * 2], BF, False)
    out = nc.declare_dram_parameter("out", [2, T], F32, True)

    with tile.TileContext(nc) as tc, ExitStack() as ctx:
        const = ctx.enter_context(tc.tile_pool(name="const", bufs=1))
        gpool = ctx.enter_context(tc.tile_pool(name="gpool", bufs=2))
        wpool = ctx.enter_context(tc.tile_pool(name="wpool", bufs=1))
        hpool = ctx.enter_context(tc.tile_pool(name="hpool", bufs=1))
        psum = ctx.enter_context(tc.tile_pool(name="psum", bufs=4, space="PSUM"))
        for _ in range(reps):
            _emit_body(nc, tc, const, gpool, wpool, hpool, psum,
                       idx, uv, verts, ident, fm, w0, b0, ws, bs, woff, out,
                       nlayers)

    nc.reset()
    nc.finalize()
    return nc


def _emit_body(nc, tc, const, gpool, wpool, hpool, psum,
               idx, uv, verts, ident, fm, w0, b0, ws, bs, woff, out,
               nlayers=6):
    # ---- Sync HWDGE queue loads, critical-path order: idx first (the
    # gathers wait on it), then ws[0] (needed by layer 1 at ~16us and the
    # only big weight DMA allowed to overlap the early gathers); ws[1:]
    # go through the Pool SWDGE queue strictly BEHIND all 16 gathers so
    # they cannot starve the gather transfers.
    idx_sb = const.tile([P, NT], I32)
    nc.sync.dma_start(out=idx_sb[:], in_=idx[:])
    wcur = [wpool.tile([P, 3 * CB * Ch], BF, name=f"wlayer{li}",
                       tag=f"wlayer{li}") for li in range(nlayers)]
    if nlayers > 0:
        nc.sync.dma_start(out=wcur[0][:], in_=ws[0])
    uv_sb = const.tile([P, 4 * NT], F32)
    nc.sync.dma_start(out=uv_sb[:], in_=uv[:])
    v_sb = const.tile([P, NT * 2], F32)
    nc.sync.dma_start(out=v_sb[:], in_=verts[:])
    id_sb = const.tile([P, P], F32)
    nc.sync.dma_start(out=id_sb[:], in_=ident[:])
    w0_sb = const.tile([P, Ch], BF)
    nc.sync.dma_start(out=w0_sb[:], in_=w0[:])
    b0_sb = const.tile([P, CB], F32)
    nc.sync.dma_start(out=b0_sb[:], in_=b0[:])
    bs_sb = const.tile([P, 6 * CB], F32)
    nc.sync.dma_start(out=bs_sb[:], in_=bs[:])
    woff_sb = const.tile([P, CB * 2], BF)
    nc.sync.dma_start(out=woff_sb[:], in_=woff[:])

    # ---- activation halo buffers; pads zeroed on the (idle) DVE up front ----
    h = [[[hpool.tile([P, SEG], BF, name=f"h{g}_{ci}_{b}", tag=f"h{g}_{ci}_{b}")
           for b in range(BPC)] for ci in range(CB)] for g in range(2)]
    for g in range(2):
        for ci in range(CB):
            for b in range(BPC):
                nc.vector.memset(h[g][ci][b][:, 0:PAD], 0.0)
                nc.vector.memset(h[g][ci][b][:, PAD + N:SEG], 0.0)

    v3 = v_sb[:].rearrange("p (j t) -> p j t", t=2)       # [128, 16, 2]

    def evict(ei, dst, ps, bias_ap):
        """ReLU+bias PSUM->SBUF eviction, alternating Act / DVE engines."""
        if ei % 2 == 0:
            nc.scalar.activation(dst, ps, AF.Relu, bias=bias_ap)
        else:
            nc.vector.tensor_scalar(
                out=dst, in0=ps, scalar1=bias_ap, scalar2=0.0,
                op0=ALU.add, op1=ALU.max)

    ev = 0

    def emit_l0(g):
        nonlocal ev
        b, s = g // NS, g % NS
        for co in range(CB):
            ps = psum.tile([P, HALF], F32, tag="mm", name=f"mm0_{g}_{co}")
            nc.tensor.matmul(
                ps[:], lhsT=w0_sb[:, co * P:(co + 1) * P], rhs=x_in[g][:],
                start=True, stop=True)
            evict(ev, h[0][co][b][:, PAD + s * HALF:PAD + (s + 1) * HALF],
                  ps[:], b0_sb[:, co:co + 1])
            ev += 1

    # ---- gather + bilinear combine + transpose into x_in, software
    # pipelined: stage A(j) = gather + Act-scale + DVE-fma of the y
    # interp; stage B(j) (emitted one tile later) = x interp + verts col
    # + PE transpose; stage C(j) (two tiles later) = PSUM->x_in copy.
    # L0 for group g follows copy(4g+3); everything trails so no engine
    # queue head ever waits on a same-queue successor's dependency.
    x_in = [const.tile([P, HALF], BF, name=f"x_in{t}") for t in range(NT // 4)]
    tmps = [None] * NT
    tps_ = [None] * NT

    def stage_a(j):
        rq = gpool.tile([P, 4 * Cf], BF, name="quad", tag="quad", bufs=16)
        nc.gpsimd.indirect_dma_start(
            out=rq[:], out_offset=None, in_=fm[:],
            in_offset=IndirectOffsetOnAxis(ap=idx_sb[:, j:j + 1], axis=0))
        uy = uv_sb[:, 0 * NT + j:0 * NT + j + 1]
        wy = uv_sb[:, 1 * NT + j:1 * NT + j + 1]
        t1 = gpool.tile([P, 2 * Cf], BF, tag="t1", bufs=3)
        nc.scalar.mul(t1[:], rq[:, 0:2 * Cf], uy)
        tmp = gpool.tile([P, 2 * Cf], BF, tag="tmp", bufs=3)
        nc.vector.scalar_tensor_tensor(
            out=tmp[:], in0=rq[:, 2 * Cf:4 * Cf], scalar=wy, in1=t1[:],
            op0=ALU.mult, op1=ALU.add)
        tmps[j] = tmp

    def stage_b(j):
        tmp = tmps[j]
        ux = uv_sb[:, 2 * NT + j:2 * NT + j + 1]
        wx = uv_sb[:, 3 * NT + j:3 * NT + j + 1]
        t2 = gpool.tile([P, Cf], BF, tag="t2", bufs=3)
        nc.scalar.mul(t2[:], tmp[:, 0:Cf], ux)
        xpre = gpool.tile([P, P], F32, tag="xpre", bufs=16)
        nc.vector.scalar_tensor_tensor(
            out=xpre[:, 0:Cf], in0=tmp[:, Cf:2 * Cf], scalar=wx,
            in1=t2[:], op0=ALU.mult, op1=ALU.add)
        nc.vector.tensor_copy(out=xpre[:, Cf:Cf + 2], in_=v3[:, j, :])
        tp = psum.tile([P, P], F32, tag="tps", bufs=2, name=f"tp{j}")
        nc.tensor.transpose(out=tp[:], in_=xpre[:], identity=id_sb[:])
        tps_[j] = tp

    def stage_c(j):
        if j % 2 == 0:
            nc.scalar.copy(
                out=x_in[j // 4][:, (j % 4) * P:(j % 4 + 1) * P], in_=tps_[j][:])
        else:
            nc.vector.tensor_copy(
                out=x_in[j // 4][:, (j % 4) * P:(j % 4 + 1) * P], in_=tps_[j][:])
        if j % 4 == 3:
            emit_l0(j // 4)

    taps = [(k, ci) for k in range(3) for ci in range(CB)]
    out_sb = [const.tile([2, HALF], F32, name=f"osb_{t}") for t in range(BPC * NS)]
    fin_q = []   # deferred final-conv blocks (software pipelining on PE)

    def fin_dma(b, s, psf):
        nonlocal ev
        ot = out_sb[b * NS + s]
        if ev % 2 == 0:
            nc.scalar.copy(out=ot[:], in_=psf[:])
        else:
            nc.vector.tensor_copy(out=ot[:], in_=psf[:])
        ev += 1
        nc.sync.dma_start(
            out=out[:, b * N + s * HALF:b * N + (s + 1) * HALF], in_=ot[:])

    def emit_fin(li, b, s):
        gout = (li + 1) % 2
        sl = slice(PAD + s * HALF, PAD + (s + 1) * HALF)
        psf = psum.tile([2, HALF], F32, tag="fin", bufs=2, name=f"fin_{b}_{s}")
        for ci in range(CB):
            nc.tensor.matmul(
                psf[:],
                lhsT=woff_sb[:, ci * 2:(ci + 1) * 2],
                rhs=h[gout][ci][b][:, sl],
                start=(ci == 0), stop=(ci == CB - 1))
        fin_dma(b, s, psf)

    def emit_layer(li, b, dil):
        """One conv layer for one batch: (s, co) blocks, 12-tap matmul
        groups, alternating evictions; the very last block interleaves
        the fused final 1x1 conv with its evictions (short tail)."""
        nonlocal ev
        gin, gout = li % 2, (li + 1) % 2
        wt = wcur[li]
        last = li == nlayers - 1
        for s in range(NS):
            last_blk = last and b == BPC - 1 and s == NS - 1
            psf = None
            prev_fin = None
            if last_blk:
                psf = psum.tile([2, HALF], F32, tag="fin", bufs=2,
                                name="fin_last")
                prev_fin = fin_q.pop(0) if fin_q else None
            dsts = []
            for co in range(CB):
                ps = psum.tile([P, HALF], F32, tag="mm",
                               name=f"mm{li}_{b}_{s}_{co}")
                for ki, (k, ci) in enumerate(taps):
                    col = (k * CB + ci) * Ch + co * P
                    off = PAD + s * HALF + (k - 1) * dil
                    nc.tensor.matmul(
                        ps[:],
                        lhsT=wt[:, col:col + P],
                        rhs=h[gin][ci][b][:, off:off + HALF],
                        start=(ki == 0), stop=(ki == 3 * CB - 1))
                dst = h[gout][co][b][:, PAD + s * HALF:PAD + (s + 1) * HALF]
                evict(ev, dst, ps[:], bs_sb[:, li * CB + co:li * CB + co + 1])
                ev += 1
                dsts.append(dst)
                if last_blk:
                    # interleave: flush the pending block, then this
                    # block's final conv one co behind the evictions so
                    # the PE never waits on an eviction
                    if co == 1 and prev_fin is not None:
                        emit_fin(*prev_fin)
                    if co >= 1:
                        nc.tensor.matmul(
                            psf[:], lhsT=woff_sb[:, (co - 1) * 2:co * 2],
                            rhs=dsts[co - 1],
                            start=(co == 1), stop=False)
            if last_blk:
                nc.tensor.matmul(
                    psf[:], lhsT=woff_sb[:, (CB - 1) * 2:CB * 2],
                    rhs=dsts[CB - 1], start=False, stop=True)
                fin_dma(b, s, psf)
            elif last:
                fin_q.append((li, b, s))
                if len(fin_q) > 1:
                    emit_fin(*fin_q.pop(0))

    # ---- main schedule: gather half-batches with the conv stack
    # interleaved so the PE FIFO always holds runnable work.  Batch 0's
    # layer 1 is emitted BEFORE batch 1's gathers/transposes; it grinds
    # ~20us of matmuls while the second half of the feed streams in.
    na = nb = nc_ = 0
    for half in range(BPC):
        while na < 8 * (half + 1):
            stage_a(na); na += 1
            if na - nb >= 2:
                stage_b(nb); nb += 1
            if nb - nc_ >= 2:
                stage_c(nc_); nc_ += 1
        while nb < na:
            stage_b(nb); nb += 1
        while nc_ < na:
            stage_c(nc_); nc_ += 1
        if half == BPC - 1:
            # remaining layer weights: Pool SWDGE queue, strictly behind
            # the gathers so they cannot starve the gather transfers
            for li in range(1, nlayers):
                nc.gpsimd.dma_start(out=wcur[li][:], in_=ws[li])
        if nlayers > 0:
            emit_layer(0, half, DILS[0])
    for li in range(1, nlayers):
        for b in range(BPC):
            emit_layer(li, b, DILS[li])
    while fin_q:
        emit_fin(*fin_q.pop(0))

    if nlayers == 0:
        for b in range(BPC):
            for s in range(NS):
                emit_fin(-1, b, s)


def shard_inputs(vertices, feature_map, w0, b0, ws, bs, w_off):
    """Build the per-core input maps (host-side repack + index precompute)."""
    F16N = np.float16
    vertices = np.asarray(vertices, np.float32)
    feature_map = np.asarray(feature_map, np.float32)
    w0r = np.ascontiguousarray(w0.reshape(D, Ch)).astype(F16N)
    b0r = np.ascontiguousarray(b0.reshape(CB, P).T, np.float32)
    wsr = np.ascontiguousarray(
        ws.reshape(6, 3, CB, P, Ch).transpose(0, 3, 1, 2, 4).reshape(6, P, 3 * CB * Ch)
    ).astype(F16N)
    bsr = np.ascontiguousarray(
        bs.reshape(6, CB, P).transpose(2, 0, 1).reshape(P, 6 * CB), np.float32)
    woffr = np.ascontiguousarray(
        w_off.reshape(CB, P, 2).transpose(1, 0, 2).reshape(P, CB * 2)).astype(F16N)
    identity = np.eye(P, dtype=np.float32)

    in_maps = []
    for c in range(NCORES):
        vb = vertices[c * BPC:(c + 1) * BPC]          # [BPC, N, 2]
        vr = np.ascontiguousarray(
            vb.reshape(BPC, N // P, P, 2).transpose(2, 0, 1, 3).reshape(P, NT * 2))
        # gather indices + bilinear weights on host (f32 = device fp path)
        coords = (vb + np.float32(1.0)) * np.float32((H - 1) / 2.0)  # [BPC,N,2]
        c0 = np.clip(np.floor(coords).astype(np.int64), 0, H - 2)
        frac = coords - c0.astype(np.float32)
        ridx = (c0[..., 0] * W + c0[..., 1]
                + (np.arange(BPC, dtype=np.int64) * (H * W))[:, None])  # [BPC,N]
        idxr = np.ascontiguousarray(
            ridx.reshape(BPC, N // P, P).transpose(2, 0, 1).reshape(P, NT)
        ).astype(np.int32)
        wy, wx = frac[..., 0], frac[..., 1]
        uvr = np.stack([1.0 - wy, wy, 1.0 - wx, wx], axis=0)  # [4,BPC,N]
        uvr = np.ascontiguousarray(
            uvr.reshape(4, BPC, N // P, P).transpose(3, 0, 1, 2).reshape(P, 4 * NT)
        ).astype(np.float32)
        # quad-packed fp16 feature map: row r = [r | r+1 | r+W | r+W+1]
        fmb = feature_map[c * BPC:(c + 1) * BPC].reshape(BPC * H * W, Cf).astype(F16N)
        R = BPC * H * W
        fmp = np.zeros((R, 4 * Cf), F16N)
        fmp[:, 0 * Cf:1 * Cf] = fmb
        fmp[:R - 1, 1 * Cf:2 * Cf] = fmb[1:]
        fmp[:R - W, 2 * Cf:3 * Cf] = fmb[W:]
        fmp[:R - W - 1, 3 * Cf:4 * Cf] = fmb[W + 1:]
        in_maps.append({
            "idx": idxr, "uv": uvr, "verts": vr, "ident": identity, "fm": fmp,
            "w0": w0r, "b0": b0r, "ws": wsr, "bs": bsr, "woff": woffr,
        })
    return in_maps


def unshard_output(results):
    outs = []
    for r in results:
        o = np.asarray(r["out"])                       # [2, T] = [ch, b*N+n]
        outs.append(o.reshape(2, BPC, N).transpose(1, 2, 0))   # [BPC, N, 2]
    return np.concatenate(outs, axis=0).astype(np.float32)


_NC_CACHE = {}


def _get_program():
    if "nc" not in _NC_CACHE:
        _NC_CACHE["nc"] = build_program()
    return _NC_CACHE["nc"]


def run(inputs, trace=False):
    nc = _get_program()
    in_maps = shard_inputs(**inputs)
    res = run_bass_kernel_spmd(nc, in_maps, list(range(NCORES)), trace=trace)
    return unshard_output(res.results), res


def kernel(**inputs) -> np.ndarray:
    out, _ = run(inputs, trace=False)
    return out


# revision 20
# speedup vs baseline: 1.0835x; 1.0245x over previous
"""SnakeHead Trainium2 kernel (fp16 matmul path, pipelined head).

Model (per batch): bilinear-sample a [256,256,126] feature map at 1024
vertices, concat the (y,x) coords -> [1024,128], 1x1 conv to 512 + ReLU,
six dilated (1,3,9,9,3,1) kernel-3 conv1d layers 512->512 + ReLU, final
1x1 conv 512->2.

Strategy: data-parallel over batch, 2 batches per NeuronCore (16/8).
Per core the kernel is tensor-engine-bound (~250us of fp16 matmul at
1 col/cycle); everything else is organized to keep the PE fed:
  - gather indices + bilinear weights are computed on HOST (they depend
    only on vertices); the first indirect gather fires as soon as the
    8KB index DMA lands.  The identity (for PE transposes) also comes
    in by DMA so the Pool engine queue holds nothing but gathers.
  - fm is host-repacked to fp16 quad rows fm[r] = [r | r+1 | r+W | r+W+1]
    (1008B per token descriptor); one 128-descriptor indirect DMA per
    128-token tile (the HW takes one index per partition per DMA).
  - the bilinear combine is split across the Act engine (scale by the
    y/x weights) and the DVE (fused multiply-add), in fp16, and the
    per-tile chains are SOFTWARE PIPELINED with one tile of skew so
    neither engine's in-order queue ever head-blocks on the other.
  - layer 0 runs per 512-token gather group; conv layers run batch-outer
    so batch 0's layer 1 starts while batch 1 is still gathering.
  - all matmuls fp16 (weights cast on host, fp32 PSUM accumulate).
  - final 1x1 conv is fused into the last conv layer per (batch, slice)
    block with per-block output DMA; the very last block interleaves
    the final-conv matmuls with the evictions so only ~2us trails the
    last conv matmul.
"""

import numpy as np
from contextlib import ExitStack

import concourse.bass as bass
import concourse.bacc as bacc
import concourse.mybir as mybir
import concourse.tile as tile
from concourse.bass import IndirectOffsetOnAxis
from concourse.bass_utils import run_bass_kernel_spmd

P = 128
B, N, H, W, Cf, Ch = 16, 1024, 256, 256, 126, 512
NCORES = 8
BPC = B // NCORES          # batches per core
T = BPC * N                # tokens per core
D = Cf + 2                 # input channels to layer 0
DILS = (1, 3, 9, 9, 3, 1)
PAD = 16                   # halo >= max dilation (9)
SEG = PAD + N + PAD        # per-batch activation columns
NT = T // P                # 128-token tiles per core (16)
CB = Ch // P               # 128-channel blocks (4)
HALF = 512                 # matmul moving-dim tile (tokens)
NS = N // HALF             # 2 (token-tile slices per batch)

F32 = mybir.dt.float32
BF = mybir.dt.float16
I32 = mybir.dt.int32
AF = mybir.ActivationFunctionType
ALU = mybir.AluOpType


def build_program(reps=1, nlayers=6):
    nc = bacc.Bacc(trn_type="TRN2", target_bir_lowering=False)

    idx = nc.declare_dram_parameter("idx", [P, NT], I32, False)
    uv = nc.declare_dram_parameter("uv", [P, 4 * NT], F32, False)
    verts = nc.declare_dram_parameter("verts", [P, NT * 2], F32, False)
    ident = nc.declare_dram_parameter("ident", [P, P], F32, False)
    fm = nc.declare_dram_parameter("fm", [BPC * H * W, 4 * Cf], BF, False)
    w0 = nc.declare_dram_parameter("w0", [P, Ch], BF, False)
    b0 = nc.declare_dram_parameter("b0", [P, CB], F32, False)
    ws = nc.declare_dram_parameter("ws", [6, P, 3 * CB * Ch], BF, False)
    bs = nc.declare_dram_parameter("bs", [P, 6 * CB], F32, False)
    woff = nc.declare_dram_parameter("woff", [P, CB * 2], BF, False)
    out = nc.declare_dram_parameter("out", [2, T], F32, True)

    with tile.TileContext(nc) as tc, ExitStack() as ctx:
        const = ctx.enter_context(tc.tile_pool(name="const", bufs=1))
        gpool = ctx.enter_context(tc.tile_pool(name="gpool", bufs=2))
        wpool = ctx.enter_context(tc.tile_pool(name="wpool", bufs=1))
        hpool = ctx.enter_context(tc.tile_pool(name="hpool", bufs=1))
        psum = ctx.enter_context(tc.tile_pool(name="psum", bufs=4, space="PSUM"))
        for _ in range(reps):
            _emit_body(nc, tc, const, gpool, wpool, hpool, psum,
                       idx, uv, verts, ident, fm, w0, b0, ws, bs, woff, out,
                       nlayers)

    nc.reset()
    nc.finalize()
    return nc


def _emit_body(nc, tc, const, gpool, wpool, hpool, psum,
               idx, uv, verts, ident, fm, w0, b0, ws, bs, woff, out,
               nlayers=6):
    # ---- Sync HWDGE queue loads, critical-path order: idx first (the
    # gathers wait on it), then ws[0] (needed by layer 1 at ~16us and the
    # only big weight DMA allowed to overlap the early gathers); ws[1:]
    # go through the Pool SWDGE queue strictly BEHIND all 16 gathers so
    # they cannot starve the gather transfers.
    idx_sb = const.tile([P, NT], I32)
    nc.sync.dma_start(out=idx_sb[:], in_=idx[:])
    wcur = [wpool.tile([P, 3 * CB * Ch], BF, name=f"wlayer{li}",
                       tag=f"wlayer{li}") for li in range(nlayers)]
    if nlayers > 0:
        nc.sync.dma_start(out=wcur[0][:], in_=ws[0])
    uv_sb = const.tile([P, 4 * NT], F32)
    nc.sync.dma_start(out=uv_sb[:], in_=uv[:])
    v_sb = const.tile([P, NT * 2], F32)
    nc.sync.dma_start(out=v_sb[:], in_=verts[:])
    id_sb = const.tile([P, P], F32)
    nc.sync.dma_start(out=id_sb[:], in_=ident[:])
    w0_sb = const.tile([P, Ch], BF)
    nc.sync.dma_start(out=w0_sb[:], in_=w0[:])
    b0_sb = const.tile([P, CB], F32)
    nc.sync.dma_start(out=b0_sb[:], in_=b0[:])
    bs_sb = const.tile([P, 6 * CB], F32)
    nc.sync.dma_start(out=bs_sb[:], in_=bs[:])
    woff_sb = const.tile([P, CB * 2], BF)
    nc.sync.dma_start(out=woff_sb[:], in_=woff[:])

    # ---- activation halo buffers; pads zeroed on the (idle) DVE up front ----
    h = [[[hpool.tile([P, SEG], BF, name=f"h{g}_{ci}_{b}", tag=f"h{g}_{ci}_{b}")
           for b in range(BPC)] for ci in range(CB)] for g in range(2)]
    for g in range(2):
        for ci in range(CB):
            for b in range(BPC):
                nc.vector.memset(h[g][ci][b][:, 0:PAD], 0.0)
                nc.vector.memset(h[g][ci][b][:, PAD + N:SEG], 0.0)

    v3 = v_sb[:].rearrange("p (j t) -> p j t", t=2)       # [128, 16, 2]

    def evict(ei, dst, ps, bias_ap):
        """ReLU+bias PSUM->SBUF eviction, alternating Act / DVE engines."""
        if ei % 2 == 0:
            nc.scalar.activation(dst, ps, AF.Relu, bias=bias_ap)
        else:
            nc.vector.tensor_scalar(
                out=dst, in0=ps, scalar1=bias_ap, scalar2=0.0,
                op0=ALU.add, op1=ALU.max)

    ev = 0

    def emit_l0(g):
        nonlocal ev
        b, s = g // NS, g % NS
        for co in range(CB):
            ps = psum.tile([P, HALF], F32, tag="mm", name=f"mm0_{g}_{co}")
            nc.tensor.matmul(
                ps[:], lhsT=w0_sb[:, co * P:(co + 1) * P], rhs=x_in[g][:],
                start=True, stop=True)
            evict(ev, h[0][co][b][:, PAD + s * HALF:PAD + (s + 1) * HALF],
                  ps[:], b0_sb[:, co:co + 1])
            ev += 1

    # ---- gather + bilinear combine + transpose into x_in, software
    # pipelined: stage A(j) = gather + Act-scale + DVE-fma of the y
    # interp; stage B(j) (emitted one tile later) = x interp + verts col
    # + PE transpose; stage C(j) (two tiles later) = PSUM->x_in copy.
    # L0 for group g follows copy(4g+3); everything trails so no engine
    # queue head ever waits on a same-queue successor's dependency.
    x_in = [const.tile([P, HALF], BF, name=f"x_in{t}") for t in range(NT // 4)]
    tmps = [None] * NT
    tps_ = [None] * NT

    rq_last = [None]

    def stage_a(j):
        rq = gpool.tile([P, 4 * Cf], BF, name="quad", tag="quad", bufs=16)
        nc.gpsimd.indirect_dma_start(
            out=rq[:], out_offset=None, in_=fm[:],
            in_offset=IndirectOffsetOnAxis(ap=idx_sb[:, j:j + 1], axis=0))
        rq_last[0] = rq
        uy = uv_sb[:, 0 * NT + j:0 * NT + j + 1]
        wy = uv_sb[:, 1 * NT + j:1 * NT + j + 1]
        t1 = gpool.tile([P, 2 * Cf], BF, tag="t1", bufs=3)
        nc.scalar.mul(t1[:], rq[:, 0:2 * Cf], uy)
        tmp = gpool.tile([P, 2 * Cf], BF, tag="tmp", bufs=3)
        nc.vector.scalar_tensor_tensor(
            out=tmp[:], in0=rq[:, 2 * Cf:4 * Cf], scalar=wy, in1=t1[:],
            op0=ALU.mult, op1=ALU.add)
        tmps[j] = tmp

    def stage_b(j):
        tmp = tmps[j]
        ux = uv_sb[:, 2 * NT + j:2 * NT + j + 1]
        wx = uv_sb[:, 3 * NT + j:3 * NT + j + 1]
        t2 = gpool.tile([P, Cf], BF, tag="t2", bufs=3)
        nc.scalar.mul(t2[:], tmp[:, 0:Cf], ux)
        xpre = gpool.tile([P, P], F32, tag="xpre", bufs=16)
        nc.vector.scalar_tensor_tensor(
            out=xpre[:, 0:Cf], in0=tmp[:, Cf:2 * Cf], scalar=wx,
            in1=t2[:], op0=ALU.mult, op1=ALU.add)
        nc.vector.tensor_copy(out=xpre[:, Cf:Cf + 2], in_=v3[:, j, :])
        tp = psum.tile([P, P], F32, tag="tps", bufs=2, name=f"tp{j}")
        nc.tensor.transpose(out=tp[:], in_=xpre[:], identity=id_sb[:])
        tps_[j] = tp

    def stage_c(j):
        if j % 2 == 0:
            nc.scalar.copy(
                out=x_in[j // 4][:, (j % 4) * P:(j % 4 + 1) * P], in_=tps_[j][:])
        else:
            nc.vector.tensor_copy(
                out=x_in[j // 4][:, (j % 4) * P:(j % 4 + 1) * P], in_=tps_[j][:])
        if j % 4 == 3:
            emit_l0(j // 4)

    taps = [(k, ci) for k in range(3) for ci in range(CB)]
    out_sb = [const.tile([2, HALF], F32, name=f"osb_{t}") for t in range(BPC * NS)]
    fin_q = []   # deferred final-conv blocks (software pipelining on PE)

    def fin_dma(b, s, psf):
        nonlocal ev
        ot = out_sb[b * NS + s]
        if ev % 2 == 0:
            nc.scalar.copy(out=ot[:], in_=psf[:])
        else:
            nc.vector.tensor_copy(out=ot[:], in_=psf[:])
        ev += 1
        nc.sync.dma_start(
            out=out[:, b * N + s * HALF:b * N + (s + 1) * HALF], in_=ot[:])

    def emit_fin(li, b, s):
        gout = (li + 1) % 2
        sl = slice(PAD + s * HALF, PAD + (s + 1) * HALF)
        psf = psum.tile([2, HALF], F32, tag="fin", bufs=2, name=f"fin_{b}_{s}")
        for ci in range(CB):
            nc.tensor.matmul(
                psf[:],
                lhsT=woff_sb[:, ci * 2:(ci + 1) * 2],
                rhs=h[gout][ci][b][:, sl],
                start=(ci == 0), stop=(ci == CB - 1))
        fin_dma(b, s, psf)

    def emit_layer(li, b, dil):
        """One conv layer for one batch: (s, co) blocks, 12-tap matmul
        groups, alternating evictions; the very last block interleaves
        the fused final 1x1 conv with its evictions (short tail)."""
        nonlocal ev
        gin, gout = li % 2, (li + 1) % 2
        wt = wcur[li]
        last = li == nlayers - 1
        for s in range(NS):
            last_blk = last and b == BPC - 1 and s == NS - 1
            psf = None
            prev_fin = None
            if last_blk:
                psf = psum.tile([2, HALF], F32, tag="fin", bufs=2,
                                name="fin_last")
                prev_fin = fin_q.pop(0) if fin_q else None
            dsts = []
            for co in range(CB):
                ps = psum.tile([P, HALF], F32, tag="mm",
                               name=f"mm{li}_{b}_{s}_{co}")
                for ki, (k, ci) in enumerate(taps):
                    col = (k * CB + ci) * Ch + co * P
                    off = PAD + s * HALF + (k - 1) * dil
                    nc.tensor.matmul(
                        ps[:],
                        lhsT=wt[:, col:col + P],
                        rhs=h[gin][ci][b][:, off:off + HALF],
                        start=(ki == 0), stop=(ki == 3 * CB - 1))
                dst = h[gout][co][b][:, PAD + s * HALF:PAD + (s + 1) * HALF]
                evict(ev, dst, ps[:], bs_sb[:, li * CB + co:li * CB + co + 1])
                ev += 1
                dsts.append(dst)
                if last_blk:
                    # interleave: flush the pending block, then this
                    # block's final conv one co behind the evictions so
                    # the PE never waits on an eviction
                    if co == 1 and prev_fin is not None:
                        emit_fin(*prev_fin)
                    if co >= 1:
                        nc.tensor.matmul(
                            psf[:], lhsT=woff_sb[:, (co - 1) * 2:co * 2],
                            rhs=dsts[co - 1],
                            start=(co == 1), stop=False)
            if last_blk:
                nc.tensor.matmul(
                    psf[:], lhsT=woff_sb[:, (CB - 1) * 2:CB * 2],
                    rhs=dsts[CB - 1], start=False, stop=True)
                fin_dma(b, s, psf)
            elif last:
                fin_q.append((li, b, s))
                if len(fin_q) > 1:
                    emit_fin(*fin_q.pop(0))

    # ---- main schedule: gather half-batches with the conv stack
    # interleaved so the PE FIFO always holds runnable work.  Batch 0's
    # layer 1 is emitted BEFORE batch 1's gathers/transposes; it grinds
    # ~20us of matmuls while the second half of the feed streams in.
    na = nb = nc_ = 0
    for half in range(BPC):
        while na < 8 * (half + 1):
            stage_a(na); na += 1
            if na - nb >= 2:
                stage_b(nb); nb += 1
            if nb - nc_ >= 2:
                stage_c(nc_); nc_ += 1
        while nb < na:
            stage_b(nb); nb += 1
        while nc_ < na:
            stage_c(nc_); nc_ += 1
        if half == BPC - 1:
            # Remaining layer weights: Pool SWDGE queue behind the
            # gathers so they cannot starve the gather transfers.  The
            # Tile list-scheduler would hoist these (their deps are
            # ready at t=0) ahead of the gathers (which wait on the idx
            # DMA), so each gets a tiny gather-dependent write into its
            # destination tile first: the WAW edge pins the DMA behind
            # the last gather.
            for li in range(1, nlayers):
                nc.gpsimd.tensor_copy(out=wcur[li][0:1, 0:2],
                                      in_=rq_last[0][0:1, 0:2])
                nc.gpsimd.dma_start(out=wcur[li][:], in_=ws[li])
        if nlayers > 0:
            emit_layer(0, half, DILS[0])
    for li in range(1, nlayers):
        for b in range(BPC):
            emit_layer(li, b, DILS[li])
    while fin_q:
        emit_fin(*fin_q.pop(0))

    if nlayers == 0:
        for b in range(BPC):
            for s in range(NS):
                emit_fin(-1, b, s)


def shard_inputs(vertices, feature_map, w0, b0, ws, bs, w_off):
    """Build the per-core input maps (host-side repack + index precompute)."""
    F16N = np.float16
    vertices = np.asarray(vertices, np.float32)
    feature_map = np.asarray(feature_map, np.float32)
    w0r = np.ascontiguousarray(w0.reshape(D, Ch)).astype(F16N)
    b0r = np.ascontiguousarray(b0.reshape(CB, P).T, np.float32)
    wsr = np.ascontiguousarray(
        ws.reshape(6, 3, CB, P, Ch).transpose(0, 3, 1, 2, 4).reshape(6, P, 3 * CB * Ch)
    ).astype(F16N)
    bsr = np.ascontiguousarray(
        bs.reshape(6, CB, P).transpose(2, 0, 1).reshape(P, 6 * CB), np.float32)
    woffr = np.ascontiguousarray(
        w_off.reshape(CB, P, 2).transpose(1, 0, 2).reshape(P, CB * 2)).astype(F16N)
    identity = np.eye(P, dtype=np.float32)

    in_maps = []
    for c in range(NCORES):
        vb = vertices[c * BPC:(c + 1) * BPC]          # [BPC, N, 2]
        vr = np.ascontiguousarray(
            vb.reshape(BPC, N // P, P, 2).transpose(2, 0, 1, 3).reshape(P, NT * 2))
        # gather indices + bilinear weights on host (f32 = device fp path)
        coords = (vb + np.float32(1.0)) * np.float32((H - 1) / 2.0)  # [BPC,N,2]
        c0 = np.clip(np.floor(coords).astype(np.int64), 0, H - 2)
        frac = coords - c0.astype(np.float32)
        ridx = (c0[..., 0] * W + c0[..., 1]
                + (np.arange(BPC, dtype=np.int64) * (H * W))[:, None])  # [BPC,N]
        idxr = np.ascontiguousarray(
            ridx.reshape(BPC, N // P, P).transpose(2, 0, 1).reshape(P, NT)
        ).astype(np.int32)
        wy, wx = frac[..., 0], frac[..., 1]
        uvr = np.stack([1.0 - wy, wy, 1.0 - wx, wx], axis=0)  # [4,BPC,N]
        uvr = np.ascontiguousarray(
            uvr.reshape(4, BPC, N // P, P).transpose(3, 0, 1, 2).reshape(P, 4 * NT)
        ).astype(np.float32)
        # quad-packed fp16 feature map: row r = [r | r+1 | r+W | r+W+1]
        fmb = feature_map[c * BPC:(c + 1) * BPC].reshape(BPC * H * W, Cf).astype(F16N)
        R = BPC * H * W
        fmp = np.zeros((R, 4 * Cf), F16N)
        fmp[:, 0 * Cf:1 * Cf] = fmb
        fmp[:R - 1, 1 * Cf:2 * Cf] = fmb[1:]
        fmp[:R - W, 2 * Cf:3 * Cf] = fmb[W:]
        fmp[:R - W - 1, 3 * Cf:4 * Cf] = fmb[W + 1:]
        in_maps.append({
            "idx": idxr, "uv": uvr, "verts": vr, "ident": identity, "fm": fmp,
            "w0": w0r, "b0": b0r, "ws": wsr, "bs": bsr, "woff": woffr,
        })
    return in_maps


def unshard_output(results):
    outs = []
    for r in results:
        o = np.asarray(r["out"])                       # [2, T] = [ch, b*N+n]
        outs.append(o.reshape(2, BPC, N).transpose(1, 2, 0))   # [BPC, N, 2]
    return np.concatenate(outs, axis=0).astype(np.float32)


_NC_CACHE = {}


def _get_program():
    if "nc" not in _NC_CACHE:
        _NC_CACHE["nc"] = build_program()
    return _NC_CACHE["nc"]


def run(inputs, trace=False):
    nc = _get_program()
    in_maps = shard_inputs(**inputs)
    res = run_bass_kernel_spmd(nc, in_maps, list(range(NCORES)), trace=trace)
    return unshard_output(res.results), res


def kernel(**inputs) -> np.ndarray:
    out, _ = run(inputs, trace=False)
    return out


# revision 21
# speedup vs baseline: 1.0843x; 1.0007x over previous
"""SnakeHead Trainium2 kernel (fp16 matmul path, pipelined head).

Model (per batch): bilinear-sample a [256,256,126] feature map at 1024
vertices, concat the (y,x) coords -> [1024,128], 1x1 conv to 512 + ReLU,
six dilated (1,3,9,9,3,1) kernel-3 conv1d layers 512->512 + ReLU, final
1x1 conv 512->2.

Strategy: data-parallel over batch, 2 batches per NeuronCore (16/8).
Per core the kernel is tensor-engine-bound (~250us of fp16 matmul at
1 col/cycle); everything else is organized to keep the PE fed:
  - gather indices + bilinear weights are computed on HOST (they depend
    only on vertices); the first indirect gather fires as soon as the
    8KB index DMA lands.  The identity (for PE transposes) also comes
    in by DMA so the Pool engine queue holds nothing but gathers.
  - fm is host-repacked to fp16 quad rows fm[r] = [r | r+1 | r+W | r+W+1]
    (1008B per token descriptor); one 128-descriptor indirect DMA per
    128-token tile (the HW takes one index per partition per DMA).
  - the bilinear combine is split across the Act engine (scale by the
    y/x weights) and the DVE (fused multiply-add), in fp16, and the
    per-tile chains are SOFTWARE PIPELINED with one tile of skew so
    neither engine's in-order queue ever head-blocks on the other.
  - layer 0 runs per 512-token gather group; conv layers run batch-outer
    so batch 0's layer 1 starts while batch 1 is still gathering.
  - all matmuls fp16 (weights cast on host, fp32 PSUM accumulate).
  - final 1x1 conv is fused into the last conv layer per (batch, slice)
    block with per-block output DMA; the very last block interleaves
    the final-conv matmuls with the evictions so only ~2us trails the
    last conv matmul.
"""

import numpy as np
from contextlib import ExitStack

import concourse.bass as bass
import concourse.bacc as bacc
import concourse.mybir as mybir
import concourse.tile as tile
from concourse.bass import IndirectOffsetOnAxis
from concourse.bass_utils import run_bass_kernel_spmd

P = 128
B, N, H, W, Cf, Ch = 16, 1024, 256, 256, 126, 512
NCORES = 8
BPC = B // NCORES          # batches per core
T = BPC * N                # tokens per core
D = Cf + 2                 # input channels to layer 0
DILS = (1, 3, 9, 9, 3, 1)
PAD = 16                   # halo >= max dilation (9)
SEG = PAD + N + PAD        # per-batch activation columns
NT = T // P                # 128-token tiles per core (16)
CB = Ch // P               # 128-channel blocks (4)
HALF = 512                 # matmul moving-dim tile (tokens)
NS = N // HALF             # 2 (token-tile slices per batch)

F32 = mybir.dt.float32
BF = mybir.dt.float16
I32 = mybir.dt.int32
AF = mybir.ActivationFunctionType
ALU = mybir.AluOpType


def build_program(reps=1, nlayers=6):
    nc = bacc.Bacc(trn_type="TRN2", target_bir_lowering=False)

    idx = nc.declare_dram_parameter("idx", [P, NT], I32, False)
    uv = nc.declare_dram_parameter("uv", [P, 4 * NT], F32, False)
    verts = nc.declare_dram_parameter("verts", [P, NT * 2], F32, False)
    ident = nc.declare_dram_parameter("ident", [P, P], F32, False)
    fm = nc.declare_dram_parameter("fm", [BPC * H * W, 4 * Cf], BF, False)
    w0 = nc.declare_dram_parameter("w0", [P, Ch], BF, False)
    b0 = nc.declare_dram_parameter("b0", [P, CB], F32, False)
    ws = nc.declare_dram_parameter("ws", [6, P, 3 * CB * Ch], BF, False)
    bs = nc.declare_dram_parameter("bs", [P, 6 * CB], F32, False)
    woff = nc.declare_dram_parameter("woff", [P, CB * 2], BF, False)
    out = nc.declare_dram_parameter("out", [2, T], F32, True)

    with tile.TileContext(nc) as tc, ExitStack() as ctx:
        const = ctx.enter_context(tc.tile_pool(name="const", bufs=1))
        gpool = ctx.enter_context(tc.tile_pool(name="gpool", bufs=2))
        wpool = ctx.enter_context(tc.tile_pool(name="wpool", bufs=1))
        hpool = ctx.enter_context(tc.tile_pool(name="hpool", bufs=1))
        psum = ctx.enter_context(tc.tile_pool(name="psum", bufs=4, space="PSUM"))
        for _ in range(reps):
            _emit_body(nc, tc, const, gpool, wpool, hpool, psum,
                       idx, uv, verts, ident, fm, w0, b0, ws, bs, woff, out,
                       nlayers)

    nc.reset()
    nc.finalize()
    return nc


def _emit_body(nc, tc, const, gpool, wpool, hpool, psum,
               idx, uv, verts, ident, fm, w0, b0, ws, bs, woff, out,
               nlayers=6):
    # ---- warmups: a dummy Relu pulls the lazy ACT_TABLE_LOAD (1.3us)
    # into the preamble instead of blocking the first combine; a tiny
    # 2-descriptor dummy gather pays the SWDGE ring's ~3us first-transfer
    # latency before the real index tile has even arrived.
    warm = const.tile([2, 4], F32)
    nc.gpsimd.memset(warm[:], 0.0)
    warm2 = const.tile([2, 4], F32)
    nc.scalar.activation(warm2[:], warm[:], AF.Relu)
    zidx = const.tile([2, 1], I32)
    nc.gpsimd.memset(zidx[:], 0)
    wrq = const.tile([2, 4 * Cf], BF)
    nc.gpsimd.indirect_dma_start(
        out=wrq[:], out_offset=None, in_=fm[:],
        in_offset=IndirectOffsetOnAxis(ap=zidx[:], axis=0))

    # ---- Sync HWDGE queue loads, critical-path order: idx first (the
    # gathers wait on it), then ws[0] (needed by layer 1 at ~16us and the
    # only big weight DMA allowed to overlap the early gathers); ws[1:]
    # go through the Pool SWDGE queue strictly BEHIND all 16 gathers so
    # they cannot starve the gather transfers.
    idx_sb = const.tile([P, NT], I32)
    nc.sync.dma_start(out=idx_sb[:], in_=idx[:])
    wcur = [wpool.tile([P, 3 * CB * Ch], BF, name=f"wlayer{li}",
                       tag=f"wlayer{li}") for li in range(nlayers)]
    if nlayers > 0:
        nc.sync.dma_start(out=wcur[0][:], in_=ws[0])
    uv_sb = const.tile([P, 4 * NT], F32)
    nc.sync.dma_start(out=uv_sb[:], in_=uv[:])
    v_sb = const.tile([P, NT * 2], F32)
    nc.sync.dma_start(out=v_sb[:], in_=verts[:])
    id_sb = const.tile([P, P], F32)
    nc.sync.dma_start(out=id_sb[:], in_=ident[:])
    w0_sb = const.tile([P, Ch], BF)
    nc.sync.dma_start(out=w0_sb[:], in_=w0[:])
    b0_sb = const.tile([P, CB], F32)
    nc.sync.dma_start(out=b0_sb[:], in_=b0[:])
    bs_sb = const.tile([P, 6 * CB], F32)
    nc.sync.dma_start(out=bs_sb[:], in_=bs[:])
    woff_sb = const.tile([P, CB * 2], BF)
    nc.sync.dma_start(out=woff_sb[:], in_=woff[:])

    # ---- activation halo buffers; pads zeroed on the (idle) DVE up front ----
    h = [[[hpool.tile([P, SEG], BF, name=f"h{g}_{ci}_{b}", tag=f"h{g}_{ci}_{b}")
           for b in range(BPC)] for ci in range(CB)] for g in range(2)]
    for g in range(2):
        for ci in range(CB):
            for b in range(BPC):
                nc.vector.memset(h[g][ci][b][:, 0:PAD], 0.0)
                nc.vector.memset(h[g][ci][b][:, PAD + N:SEG], 0.0)

    v3 = v_sb[:].rearrange("p (j t) -> p j t", t=2)       # [128, 16, 2]

    def evict(ei, dst, ps, bias_ap):
        """ReLU+bias PSUM->SBUF eviction, alternating Act / DVE engines."""
        if ei % 2 == 0:
            nc.scalar.activation(dst, ps, AF.Relu, bias=bias_ap)
        else:
            nc.vector.tensor_scalar(
                out=dst, in0=ps, scalar1=bias_ap, scalar2=0.0,
                op0=ALU.add, op1=ALU.max)

    ev = 0

    def emit_l0(g):
        nonlocal ev
        b, s = g // NS, g % NS
        for co in range(CB):
            ps = psum.tile([P, HALF], F32, tag="mm", name=f"mm0_{g}_{co}")
            nc.tensor.matmul(
                ps[:], lhsT=w0_sb[:, co * P:(co + 1) * P], rhs=x_in[g][:],
                start=True, stop=True)
            evict(ev, h[0][co][b][:, PAD + s * HALF:PAD + (s + 1) * HALF],
                  ps[:], b0_sb[:, co:co + 1])
            ev += 1

    # ---- gather + bilinear combine + transpose into x_in, software
    # pipelined: stage A(j) = gather + Act-scale + DVE-fma of the y
    # interp; stage B(j) (emitted one tile later) = x interp + verts col
    # + PE transpose; stage C(j) (two tiles later) = PSUM->x_in copy.
    # L0 for group g follows copy(4g+3); everything trails so no engine
    # queue head ever waits on a same-queue successor's dependency.
    x_in = [const.tile([P, HALF], BF, name=f"x_in{t}") for t in range(NT // 4)]
    tmps = [None] * NT
    tps_ = [None] * NT

    rq_last = [None]

    def stage_a(j):
        rq = gpool.tile([P, 4 * Cf], BF, name="quad", tag="quad", bufs=16)
        nc.gpsimd.indirect_dma_start(
            out=rq[:], out_offset=None, in_=fm[:],
            in_offset=IndirectOffsetOnAxis(ap=idx_sb[:, j:j + 1], axis=0))
        rq_last[0] = rq
        uy = uv_sb[:, 0 * NT + j:0 * NT + j + 1]
        wy = uv_sb[:, 1 * NT + j:1 * NT + j + 1]
        t1 = gpool.tile([P, 2 * Cf], BF, tag="t1", bufs=3)
        nc.scalar.mul(t1[:], rq[:, 0:2 * Cf], uy)
        tmp = gpool.tile([P, 2 * Cf], BF, tag="tmp", bufs=3)
        nc.vector.scalar_tensor_tensor(
            out=tmp[:], in0=rq[:, 2 * Cf:4 * Cf], scalar=wy, in1=t1[:],
            op0=ALU.mult, op1=ALU.add)
        tmps[j] = tmp

    def stage_b(j):
        tmp = tmps[j]
        ux = uv_sb[:, 2 * NT + j:2 * NT + j + 1]
        wx = uv_sb[:, 3 * NT + j:3 * NT + j + 1]
        t2 = gpool.tile([P, Cf], BF, tag="t2", bufs=3)
        nc.scalar.mul(t2[:], tmp[:, 0:Cf], ux)
        xpre = gpool.tile([P, P], F32, tag="xpre", bufs=16)
        nc.vector.scalar_tensor_tensor(
            out=xpre[:, 0:Cf], in0=tmp[:, Cf:2 * Cf], scalar=wx,
            in1=t2[:], op0=ALU.mult, op1=ALU.add)
        nc.vector.tensor_copy(out=xpre[:, Cf:Cf + 2], in_=v3[:, j, :])
        tp = psum.tile([P, P], F32, tag="tps", bufs=2, name=f"tp{j}")
        nc.tensor.transpose(out=tp[:], in_=xpre[:], identity=id_sb[:])
        tps_[j] = tp

    def stage_c(j):
        if j % 2 == 0:
            nc.scalar.copy(
                out=x_in[j // 4][:, (j % 4) * P:(j % 4 + 1) * P], in_=tps_[j][:])
        else:
            nc.vector.tensor_copy(
                out=x_in[j // 4][:, (j % 4) * P:(j % 4 + 1) * P], in_=tps_[j][:])
        if j % 4 == 3:
            emit_l0(j // 4)

    taps = [(k, ci) for k in range(3) for ci in range(CB)]
    out_sb = [const.tile([2, HALF], F32, name=f"osb_{t}") for t in range(BPC * NS)]
    fin_q = []   # deferred final-conv blocks (software pipelining on PE)

    def fin_dma(b, s, psf):
        nonlocal ev
        ot = out_sb[b * NS + s]
        if ev % 2 == 0:
            nc.scalar.copy(out=ot[:], in_=psf[:])
        else:
            nc.vector.tensor_copy(out=ot[:], in_=psf[:])
        ev += 1
        nc.sync.dma_start(
            out=out[:, b * N + s * HALF:b * N + (s + 1) * HALF], in_=ot[:])

    def emit_fin(li, b, s):
        gout = (li + 1) % 2
        sl = slice(PAD + s * HALF, PAD + (s + 1) * HALF)
        psf = psum.tile([2, HALF], F32, tag="fin", bufs=2, name=f"fin_{b}_{s}")
        for ci in range(CB):
            nc.tensor.matmul(
                psf[:],
                lhsT=woff_sb[:, ci * 2:(ci + 1) * 2],
                rhs=h[gout][ci][b][:, sl],
                start=(ci == 0), stop=(ci == CB - 1))
        fin_dma(b, s, psf)

    def emit_layer(li, b, dil):
        """One conv layer for one batch: (s, co) blocks, 12-tap matmul
        groups, alternating evictions; the very last block interleaves
        the fused final 1x1 conv with its evictions (short tail)."""
        nonlocal ev
        gin, gout = li % 2, (li + 1) % 2
        wt = wcur[li]
        last = li == nlayers - 1
        for s in range(NS):
            last_blk = last and b == BPC - 1 and s == NS - 1
            psf = None
            prev_fin = None
            if last_blk:
                psf = psum.tile([2, HALF], F32, tag="fin", bufs=2,
                                name="fin_last")
                prev_fin = fin_q.pop(0) if fin_q else None
            dsts = []
            for co in range(CB):
                ps = psum.tile([P, HALF], F32, tag="mm",
                               name=f"mm{li}_{b}_{s}_{co}")
                for ki, (k, ci) in enumerate(taps):
                    col = (k * CB + ci) * Ch + co * P
                    off = PAD + s * HALF + (k - 1) * dil
                    nc.tensor.matmul(
                        ps[:],
                        lhsT=wt[:, col:col + P],
                        rhs=h[gin][ci][b][:, off:off + HALF],
                        start=(ki == 0), stop=(ki == 3 * CB - 1))
                dst = h[gout][co][b][:, PAD + s * HALF:PAD + (s + 1) * HALF]
                evict(ev, dst, ps[:], bs_sb[:, li * CB + co:li * CB + co + 1])
                ev += 1
                dsts.append(dst)
                if last_blk:
                    # interleave: flush the pending block, then this
                    # block's final conv one co behind the evictions so
                    # the PE never waits on an eviction
                    if co == 1 and prev_fin is not None:
                        emit_fin(*prev_fin)
                    if co >= 1:
                        nc.tensor.matmul(
                            psf[:], lhsT=woff_sb[:, (co - 1) * 2:co * 2],
                            rhs=dsts[co - 1],
                            start=(co == 1), stop=False)
            if last_blk:
                nc.tensor.matmul(
                    psf[:], lhsT=woff_sb[:, (CB - 1) * 2:CB * 2],
                    rhs=dsts[CB - 1], start=False, stop=True)
                fin_dma(b, s, psf)
            elif last:
                fin_q.append((li, b, s))
                if len(fin_q) > 1:
                    emit_fin(*fin_q.pop(0))

    # ---- main schedule: gather half-batches with the conv stack
    # interleaved so the PE FIFO always holds runnable work.  Batch 0's
    # layer 1 is emitted BEFORE batch 1's gathers/transposes; it grinds
    # ~20us of matmuls while the second half of the feed streams in.
    na = nb = nc_ = 0
    for half in range(BPC):
        while na < 8 * (half + 1):
            stage_a(na); na += 1
            if na - nb >= 2:
                stage_b(nb); nb += 1
            if nb - nc_ >= 2:
                stage_c(nc_); nc_ += 1
        while nb < na:
            stage_b(nb); nb += 1
        while nc_ < na:
            stage_c(nc_); nc_ += 1
        if half == BPC - 1:
            # Remaining layer weights: Pool SWDGE queue behind the
            # gathers so they cannot starve the gather transfers.  The
            # Tile list-scheduler would hoist these (their deps are
            # ready at t=0) ahead of the gathers (which wait on the idx
            # DMA), so each gets a tiny gather-dependent write into its
            # destination tile first: the WAW edge pins the DMA behind
            # the last gather.
            for li in range(1, nlayers):
                nc.gpsimd.tensor_copy(out=wcur[li][0:1, 0:2],
                                      in_=rq_last[0][0:1, 0:2])
                nc.gpsimd.dma_start(out=wcur[li][:], in_=ws[li])
        if nlayers > 0:
            emit_layer(0, half, DILS[0])
    for li in range(1, nlayers):
        for b in range(BPC):
            emit_layer(li, b, DILS[li])
    while fin_q:
        emit_fin(*fin_q.pop(0))

    if nlayers == 0:
        for b in range(BPC):
            for s in range(NS):
                emit_fin(-1, b, s)


def shard_inputs(vertices, feature_map, w0, b0, ws, bs, w_off):
    """Build the per-core input maps (host-side repack + index precompute)."""
    F16N = np.float16
    vertices = np.asarray(vertices, np.float32)
    feature_map = np.asarray(feature_map, np.float32)
    w0r = np.ascontiguousarray(w0.reshape(D, Ch)).astype(F16N)
    b0r = np.ascontiguousarray(b0.reshape(CB, P).T, np.float32)
    wsr = np.ascontiguousarray(
        ws.reshape(6, 3, CB, P, Ch).transpose(0, 3, 1, 2, 4).reshape(6, P, 3 * CB * Ch)
    ).astype(F16N)
    bsr = np.ascontiguousarray(
        bs.reshape(6, CB, P).transpose(2, 0, 1).reshape(P, 6 * CB), np.float32)
    woffr = np.ascontiguousarray(
        w_off.reshape(CB, P, 2).transpose(1, 0, 2).reshape(P, CB * 2)).astype(F16N)
    identity = np.eye(P, dtype=np.float32)

    in_maps = []
    for c in range(NCORES):
        vb = vertices[c * BPC:(c + 1) * BPC]          # [BPC, N, 2]
        vr = np.ascontiguousarray(
            vb.reshape(BPC, N // P, P, 2).transpose(2, 0, 1, 3).reshape(P, NT * 2))
        # gather indices + bilinear weights on host (f32 = device fp path)
        coords = (vb + np.float32(1.0)) * np.float32((H - 1) / 2.0)  # [BPC,N,2]
        c0 = np.clip(np.floor(coords).astype(np.int64), 0, H - 2)
        frac = coords - c0.astype(np.float32)
        ridx = (c0[..., 0] * W + c0[..., 1]
                + (np.arange(BPC, dtype=np.int64) * (H * W))[:, None])  # [BPC,N]
        idxr = np.ascontiguousarray(
            ridx.reshape(BPC, N // P, P).transpose(2, 0, 1).reshape(P, NT)
        ).astype(np.int32)
        wy, wx = frac[..., 0], frac[..., 1]
        uvr = np.stack([1.0 - wy, wy, 1.0 - wx, wx], axis=0)  # [4,BPC,N]
        uvr = np.ascontiguousarray(
            uvr.reshape(4, BPC, N // P, P).transpose(3, 0, 1, 2).reshape(P, 4 * NT)
        ).astype(np.float32)
        # quad-packed fp16 feature map: row r = [r | r+1 | r+W | r+W+1]
        fmb = feature_map[c * BPC:(c + 1) * BPC].reshape(BPC * H * W, Cf).astype(F16N)
        R = BPC * H * W
        fmp = np.zeros((R, 4 * Cf), F16N)
        fmp[:, 0 * Cf:1 * Cf] = fmb
        fmp[:R - 1, 1 * Cf:2 * Cf] = fmb[1:]
        fmp[:R - W, 2 * Cf:3 * Cf] = fmb[W:]
        fmp[:R - W - 1, 3 * Cf:4 * Cf] = fmb[W + 1:]
        in_maps.append({
            "idx": idxr, "uv": uvr, "verts": vr, "ident": identity, "fm": fmp,
            "w0": w0r, "b0": b0r, "ws": wsr, "bs": bsr, "woff": woffr,
        })
    return in_maps


def unshard_output(results):
    outs = []
    for r in results:
        o = np.asarray(r["out"])                       # [2, T] = [ch, b*N+n]
        outs.append(o.reshape(2, BPC, N).transpose(1, 2, 0))   # [BPC, N, 2]
    return np.concatenate(outs, axis=0).astype(np.float32)


_NC_CACHE = {}


def _get_program():
    if "nc" not in _NC_CACHE:
        _NC_CACHE["nc"] = build_program()
    return _NC_CACHE["nc"]


def run(inputs, trace=False):
    nc = _get_program()
    in_maps = shard_inputs(**inputs)
    res = run_bass_kernel_spmd(nc, in_maps, list(range(NCORES)), trace=trace)
    return unshard_output(res.results), res


def kernel(**inputs) -> np.ndarray:
    out, _ = run(inputs, trace=False)
    return out
